# revision 24
# baseline (speedup 1.0000x reference)
"""Trainium2 Bass kernel for a 16-head causal MHA layer.

Problem: x:[2,2048,1024] f32, wq/wk/wv/wo:[1024,1024] f32 (Linear-style
[out,in] weights), causal softmax attention with 16 heads of dim 64.

Sharding across the 8 NeuronCores: 2-way data parallel over batch x
4-way tensor parallel over heads.  Core c handles batch c//4 and the 4
heads 4*(c%4) .. 4*(c%4)+3 (feature slice of 256 rows of wq/wk/wv and
256 columns of wo).  Each core produces a partial [2048,1024] output
(its 4 heads' contribution, already projected through its wo slice);
the host sums the 4 partials per batch.

Device dataflow (all matmul inputs fp16, fp32 PSUM accumulation; fp8
was tried and measured 2.1% l2 error -- the softmax does not attenuate
relative error since the attention output shrinks by the same
sqrt(eff_k) factor as the injected score noise -- so everything stays
fp16):
  - x arrives fp16, striped by 512-token chunks, first stripe split
    across both hardware DMA queues so projections start early
  - qT/kT = W @ xT in [feat, token] layout; the reference 1/sqrt(64)
    score scale is folded into the exp activation's free scale slot
  - scoresT[k,q] = kT_h.T-block @ qT_h (64-dim contraction, two heads
    packed onto PE row-halves via tile_position), exp on ACT straight
    out of PSUM, causal mask applied only on diagonal blocks via a
    precomputed 0/1 mask multiply
  - out_unnorm.T | l = (v|1).T-block @ expT accumulated over k blocks
    (the appended ones-column yields the softmax denominator l for free)
  - the whole attention runs as one flat pipeline of (g,half) steps:
    each step's score matmuls are issued one step AHEAD of the exp/AV
    work so the exp engine (the attention pacer at ~2.3us/step vs
    ~1.3us of PE work) never waits on the Tensor queue; projection /
    normalize / output-projection work drains into the leftover PE
    slack from two priority deques (proj chunks must finish before the
    next q-chunk's scores; norm/wo are elastic)
  - 1/l via a DRAM-roundtrip transpose to [128,x] + DVE reciprocal
    (a [1,512] single-lane reciprocal measures 3.3us vs 0.17us for the
    [128,4] layout), broadcast over the dh rows with K=128 ident
    matmuls, two heads packed onto PE column-halves; for the final
    q-chunk the roundtrip latency would sit on the critical tail, so
    l is transposed on-chip with [1,128]->[128,1] PE transposes instead
  - y = outT.T @ woT accumulated over the 256-dim feature slice; the
    two 512-wide output halves share one [128,1024] staging tile and a
    single DMA per 128-token row block; the tail blocks' PSUM->SBUF
    casts are split across Vector and the (by then idle) Scalar engine
"""

import numpy as np

S = 2048          # sequence length (one batch per core)
D = 1024          # model dim
HL = 4            # heads handled per core
DH = 64           # head dim
F = HL * DH       # 256 local features
DC = D // 128     # 8 d_model chunks of 128
FC = F // 128     # 2 feature chunks of 128
NT = S // 128     # 16 token tiles
NQ = S // 512     # 4 query chunks of 512

_CACHE = {}


def _build_program(dbg=False):
    key = ("nc", dbg)
    if key in _CACHE:
        return _CACHE[key]

    import collections

    import concourse.bacc as bacc
    import concourse.bass as bass
    import concourse.mybir as mybir
    import concourse.tile as tile

    f16 = mybir.dt.float16
    f32 = mybir.dt.float32
    Exp = mybir.ActivationFunctionType.Exp

    nc = bacc.Bacc("TRN2", target_bir_lowering=False, debug=False)

    # x striped by 512-token chunks: xT_d[t5][p, dc, j] = x[t5*512+j, dc*128+p]
    xT_d = nc.dram_tensor("xT", [NQ, 128, DC, 512], f16, kind="ExternalInput")
    wq_d = nc.dram_tensor("wq", [128, DC, F], f16, kind="ExternalInput")
    wk_d = nc.dram_tensor("wk", [128, DC, F], f16, kind="ExternalInput")
    wv_d = nc.dram_tensor("wv", [128, DC, F], f16, kind="ExternalInput")
    wo_d = nc.dram_tensor("wo", [128, FC, D], f16, kind="ExternalInput")
    mask_d = nc.dram_tensor("mask", [128, 896], f16, kind="ExternalInput")
    ident_d = nc.dram_tensor("ident", [128, 128], f16, kind="ExternalInput")
    y_d = nc.dram_tensor("y", [S, D], f16, kind="ExternalOutput")

    with tile.TileContext(nc) as tc:
        with tc.tile_pool(name="const", bufs=1) as cpool, \
             tc.tile_pool(name="dscr", bufs=1,
                          space=bass.MemorySpace.DRAM) as dpool:
            l_dram = dpool.tile([HL * S], f32)
            xT = cpool.tile([128, NQ, DC, 512], f16)
            wq = cpool.tile([128, DC, F], f16)
            wk = cpool.tile([128, DC, F], f16)
            wv = cpool.tile([128, DC, F], f16)
            wo = cpool.tile([128, FC, D], f16)
            mask = cpool.tile([128, 896], f16)
            ident = cpool.tile([128, 128], f16)
            qT = cpool.tile([128, FC, S], f16)
            kT = cpool.tile([128, FC, S], f16)
            v = cpool.tile([128, NT, HL, DH + 1], f16)
            outT = cpool.tile([128, FC, S], f16)
            l_row = cpool.tile([1, HL * S], f32)
            lT = cpool.tile([128, HL * NT], f32)
            recipT16 = cpool.tile([128, HL * NT], f16)
            ones1 = cpool.tile([1, 1], f32)

            # loads: the DMA rings round-robin bandwidth across ALL
            # queued transfers, so anything queued early steals from the
            # critical path.  Only the first-needed 3MB goes on the two
            # hardware rings up front (sync: x stripes 0-1, scalar: wq+wk);
            # wv/mask/ident ride the gpsimd software-DGE path, and the
            # stragglers (x stripes 2-3, wo) are emitted later at natural
            # staging points so their transfers start late.
            nc.sync.dma_start(xT[:, 0], xT_d[0])
            nc.scalar.dma_start(wq[:], wq_d[:])
            nc.sync.dma_start(xT[:, 1], xT_d[1])
            nc.gpsimd.dma_start(wv[:], wv_d[:])
            nc.gpsimd.dma_start(mask[:], mask_d[:])
            nc.gpsimd.dma_start(ident[:], ident_d[:])

            # constants / ones columns for the softmax-denominator trick
            nc.gpsimd.memset(v[:], 1.0)
            nc.gpsimd.memset(ones1[:], 1.0)

            with tc.tile_pool(name="sc_ps", bufs=2,
                              space=bass.MemorySpace.PSUM) as scp, \
                 tc.tile_pool(name="av_ps", bufs=2,
                              space=bass.MemorySpace.PSUM) as avp, \
                 tc.tile_pool(name="ybc_ps", bufs=2,
                              space=bass.MemorySpace.PSUM) as ybcp, \
                 tc.tile_pool(name="p_sb", bufs=6) as ppool, \
                 tc.tile_pool(name="y_sb", bufs=3) as ysb_pool:

                # HAM warmup: dummy matmuls during the input-load window so
                # the PE clock-gate is at 8/8 when real work arrives; also
                # pre-trigger the exp ACT table load off the critical path.
                warm = ppool.tile([128, 128], f16, tag="warm", bufs=1)
                warm2 = ppool.tile([128, 128], f16, tag="warm2", bufs=1)
                nc.vector.memset(warm[:], 1.0)
                nc.scalar.dma_start(wk[:], wk_d[:])
                nc.scalar.activation(warm2[:, 0:1], warm[:, 0:1], Exp)
                wps = ybcp.tile([128, 512], f32, tag="ybc", name="warm_ps")
                for _ in range(8):
                    nc.tensor.matmul(
                        wps[:], warm[:],
                        warm[:, 0:1].to_broadcast((128, 512)),
                        start=True, stop=True)

                # quarter-size projection chunks keep the filler
                # granularity near ~0.4us so the per-step drain slots pack
                # the PE slack left by the exp-paced attention pipeline
                NCH = 4

                def proj_qk_chunk(w_sb, dstT, fc, t5, ch, state):
                    if ch == 0:
                        state[fc] = ybcp.tile([128, 512], f32, tag="ybc",
                                              name=f"ps_{t5}_{fc}")
                    ps = state[fc]
                    per = DC // NCH
                    for dc in range(per * ch, per * ch + per):
                        nc.tensor.matmul(
                            ps[:],
                            w_sb[:, dc, fc * 128:(fc + 1) * 128],
                            xT[:, t5, dc, :],
                            start=(dc == 0), stop=(dc == DC - 1))
                    if ch == NCH - 1:
                        nc.vector.tensor_copy(
                            dstT[:, fc, t5 * 512:(t5 + 1) * 512], ps[:])

                def proj_qk(t5):
                    st = {}
                    for w_sb, dstT in ((wq, qT), (wk, kT)):
                        for fc in range(FC):
                            for ch in range(NCH):
                                proj_qk_chunk(w_sb, dstT, fc, t5, ch, st)

                def proj_v_chunk(tt, ch, state):
                    t5, r = divmod(tt, 4)
                    if ch == 0:
                        state[tt] = ybcp.tile([128, F], f32, tag="ybc",
                                              name=f"psv_{tt}")
                    psv = state[tt]
                    for dc in range(4 * ch, 4 * ch + 4):
                        nc.tensor.matmul(
                            psv[:],
                            xT[:, t5, dc, r * 128:(r + 1) * 128],
                            wv[:, dc, :],
                            start=(dc == 0), stop=(dc == DC - 1))
                    if ch == 1:
                        nc.vector.tensor_copy(
                            v[:, tt, :, 0:DH],
                            psv.rearrange("p (h d) -> p h d", h=HL))

                def proj_v_group(tt):
                    st = {}
                    proj_v_chunk(tt, 0, st)
                    proj_v_chunk(tt, 1, st)

                # two priority classes: "must" fillers gate the next
                # q-chunk's scores (projections); "soft" are elastic
                must = collections.deque()
                soft = collections.deque()

                def run_filler(n):
                    for _ in range(n):
                        if must:
                            must.popleft()()
                        elif soft:
                            soft.popleft()()

                def drain_must():
                    while must:
                        must.popleft()()

                def norm_pair(qc, hc):
                    # 1/l on the [q-partition] transposed copy (128 DVE
                    # lanes), broadcast over the dh rows with K=128 ident
                    # matmuls, two heads packed onto PE column halves,
                    # then one tensor_mul normalizes the [128,512] chunk
                    sl = slice(qc * 512, (qc + 1) * 512)
                    if qc == NQ - 1:
                        # on-chip l transpose: [1,128] -> [128,1] PE
                        # transposes into PSUM; skips the DRAM roundtrip
                        # latency that would sit on the critical tail
                        ltp = ybcp.tile([128, 8], f32, tag="ybc",
                                        name=f"ltp_{hc}")
                        for hp2 in range(2):
                            h = hc * 2 + hp2
                            for t4 in range(4):
                                seg = slice(h * S + qc * 512 + t4 * 128,
                                            h * S + qc * 512 + (t4 + 1) * 128)
                                nc.tensor.transpose(
                                    ltp[:, 4 * hp2 + t4:4 * hp2 + t4 + 1],
                                    l_row[0:1, seg], ones1[:])
                        with nc.allow_low_precision(
                                reason="fp16 1/l; l>=1 so ~5e-4 relative"):
                            for hp2 in range(2):
                                h = hc * 2 + hp2
                                cols = slice(h * NT + 4 * qc,
                                             h * NT + 4 * qc + 4)
                                nc.vector.reciprocal(
                                    recipT16[:, cols],
                                    ltp[:, 4 * hp2:4 * hp2 + 4])
                    else:
                        with nc.allow_low_precision(
                                reason="fp16 1/l; l>=1 so ~5e-4 relative"):
                            for hp2 in range(2):
                                h = hc * 2 + hp2
                                cols = slice(h * NT + 4 * qc,
                                             h * NT + 4 * qc + 4)
                                nc.vector.reciprocal(recipT16[:, cols],
                                                     lT[:, cols])
                    bc = ybcp.tile([128, 512], f32, tag="ybc",
                                   name=f"bc_{hc}_{qc}")
                    for hp2 in range(2):
                        for t4 in range(4):
                            col = (hc * 2 + hp2) * NT + 4 * qc + t4
                            nc.tensor.matmul(
                                bc[hp2 * 64:(hp2 + 1) * 64,
                                   t4 * 128:(t4 + 1) * 128],
                                recipT16[:, col:col + 1]
                                .to_broadcast((128, DH)),
                                ident[:],
                                start=True, stop=True,
                                tile_position=(0, hp2 * 64))
                    nc.vector.tensor_mul(
                        outT[:, hc, sl], outT[:, hc, sl], bc[:])

                def wo_qt(qt, tail=False):
                    ysb = ysb_pool.tile([128, 1024], f16, tag="ysb",
                                        name=f"ysb_{qt}")
                    for oc in range(2):
                        yps = ybcp.tile([128, 512], f32, tag="ybc",
                                        name=f"yps_{qt}_{oc}")
                        for fc in range(FC):
                            nc.tensor.matmul(
                                yps[:],
                                outT[:, fc, qt * 128:(qt + 1) * 128],
                                wo[:, fc, oc * 512:(oc + 1) * 512],
                                start=(fc == 0), stop=(fc == FC - 1))
                        dst = ysb[:, oc * 512:(oc + 1) * 512]
                        if tail and oc == 0:
                            # scalar is idle once the exps are done; split
                            # the tail casts so Vector isn't the pacer
                            nc.scalar.copy(dst, yps[:])
                        else:
                            nc.vector.tensor_copy(dst, yps[:])
                    nc.sync.dma_start(
                        y_d[qt * 128:(qt + 1) * 128, :], ysb[:])

                # ---- attention step pipeline ---------------------------
                # one step = one 128-key block for BOTH heads of the pair:
                # 2 score matmuls (PE row-halves) into disjoint column
                # halves of ONE [128,1024] PSUM tile, one exp ACT over a
                # [128,2,w] view, mask on diagonal, 2 AV matmuls.  Scores
                # are issued one step AHEAD of the exp/AV work, and the
                # 2-deep sc pool holds exactly the in-flight step + the
                # lookahead step, so the exp engine (the attention pacer)
                # never waits on the Tensor queue.
                steps = []
                pre_fill = {}     # step idx -> (must list, soft list)
                drain_before = set()

                def build_unit(qc, hc):
                    state = {}

                    def ensure_avs():
                        if 'avs' not in state:
                            state['avs'] = [
                                avp.tile([DH + 1, 512], f32, tag="av",
                                         name=f"av_{hc}_{qc}_{hp2}")
                                for hp2 in range(2)]
                        return state['avs']

                    def finalize_early():
                        # columns [0:384] are final once kb = n_kb-2 is
                        # accumulated (the last, r=3 block only writes
                        # [384:512]); copy them under the last AV
                        avs = state['avs']
                        for hp2 in range(2):
                            h = hc * 2 + hp2
                            nc.vector.tensor_copy(
                                outT[hp2 * 64:hp2 * 64 + 64, hc,
                                     qc * 512:qc * 512 + 384],
                                avs[hp2][0:DH, 0:384])
                            seg = slice(h * S + qc * 512,
                                        h * S + qc * 512 + 384)
                            nc.vector.tensor_copy(
                                l_row[0:1, seg], avs[hp2][DH:DH + 1, 0:384])

                    def finalize():
                        # one small staging read releases the avs PSUM pair
                        # (~0.26us after its last AV); the outT/l fan-out
                        # copies run off the release path
                        avs = state['avs']
                        for hp2 in range(2):
                            h = hc * 2 + hp2
                            stg = ppool.tile([DH + 1, 128], f32,
                                             tag="stg", bufs=2,
                                             name=f"stg_{hc}_{qc}_{hp2}")
                            nc.vector.tensor_copy(stg[:],
                                                  avs[hp2][:, 384:512])
                            nc.vector.tensor_copy(
                                outT[hp2 * 64:hp2 * 64 + 64, hc,
                                     qc * 512 + 384:(qc + 1) * 512],
                                stg[0:DH, :])
                            seg = slice(h * S + qc * 512 + 384,
                                        h * S + (qc + 1) * 512)
                            nc.vector.tensor_copy(
                                l_row[0:1, seg], stg[DH:DH + 1, :])
                            if qc < NQ - 1:
                                seg = slice(h * S + qc * 512,
                                            h * S + (qc + 1) * 512)
                                nc.sync.dma_start(l_dram[seg],
                                                  l_row[0:1, seg])
                                nc.sync.dma_start(
                                    lT[:, h * NT + 4 * qc:
                                       h * NT + 4 * qc + 4],
                                    l_dram[seg]
                                    .rearrange("(t p) -> p t", p=128))
                        if qc == 0 and hc == 0:
                            # emitted here so the sync ring starts these
                            # only after the ring has drained stripes 0-1
                            nc.sync.dma_start(xT[:, 2], xT_d[2])
                            nc.sync.dma_start(xT[:, 3], xT_d[3])

                    n_kb = 4 * (qc + 1)
                    for kb in range(n_kb):
                        r = kb - 4 * qc
                        if r >= 0:
                            qo, w = 128 * r, 512 - 128 * r
                        else:
                            qo, w = 0, 512

                        def scores_fn(kb=kb, qo=qo, w=w):
                            sc = scp.tile([128, 1024], f32, tag="sc",
                                          name=f"sc_{hc}_{qc}_{kb}")
                            for hp2 in range(2):
                                hp = hp2 * 64
                                nc.tensor.matmul(
                                    sc[:, hp2 * 512 + qo:
                                       hp2 * 512 + qo + w],
                                    kT[hp:hp + 64, hc,
                                       kb * 128:(kb + 1) * 128],
                                    qT[hp:hp + 64, hc,
                                       qc * 512 + qo:(qc + 1) * 512],
                                    start=True, stop=True,
                                    tile_position=(hp, 0))
                            state[kb] = sc

                        def rest_fn(kb=kb, qo=qo, w=w, diag=(r >= 0),
                                    last=(kb == n_kb - 1)):
                            sc = state.pop(kb)
                            avs = ensure_avs()
                            p_sb = ppool.tile([128, 1024], f16, tag="p",
                                              name=f"p_{hc}_{qc}_{kb}")
                            sc2 = sc.rearrange("p (h w) -> p h w", h=2)
                            p2 = p_sb.rearrange("p (h w) -> p h w", h=2)
                            # the reference 1/sqrt(64) score scale
                            nc.scalar.activation(
                                p2[:, :, qo:qo + w],
                                sc2[:, :, qo:qo + w], Exp,
                                scale=0.125)
                            if diag:
                                # only the first 128 columns of a clipped
                                # block straddle the diagonal
                                for hp2 in range(2):
                                    o = hp2 * 512 + qo
                                    nc.vector.tensor_mul(
                                        p_sb[:, o:o + 128],
                                        p_sb[:, o:o + 128],
                                        mask[:, 384:512])
                            for hp2 in range(2):
                                h = hc * 2 + hp2
                                nc.tensor.matmul(
                                    avs[hp2][:, qo:512],
                                    v[:, kb, h, :],
                                    p_sb[:, hp2 * 512 + qo:
                                         hp2 * 512 + qo + w],
                                    start=(kb == 0),
                                    stop=(kb == n_kb - 1))
                            if kb == n_kb - 2:
                                finalize_early()
                            if last:
                                finalize()

                        steps.append((scores_fn, rest_fn))

                for qc in range(NQ):
                    idx = len(steps)
                    m, s = [], []
                    if qc + 1 < NQ:
                        st = {}
                        for w_sb, dstT in ((wq, qT), (wk, kT)):
                            for fc in range(FC):
                                for ch in range(NCH):
                                    m.append(
                                        lambda w=w_sb, d=dstT, f=fc,
                                        t=qc + 1, c=ch, stt=st:
                                        proj_qk_chunk(w, d, f, t, c, stt))
                        stv = {}
                        for tt in range(4 * (qc + 1), 4 * (qc + 2)):
                            for ch in range(2):
                                m.append(lambda t=tt, c=ch, stt=stv:
                                         proj_v_chunk(t, c, stt))
                    if qc >= 1:
                        for hcx in range(FC):
                            s.append(
                                lambda q=qc - 1, c=hcx: norm_pair(q, c))
                        for qt in range(4 * (qc - 1), 4 * qc):
                            s.append(lambda a=qt: wo_qt(a))
                    pre_fill[idx] = (m, s)
                    if qc > 0:
                        drain_before.add(idx)
                    build_unit(qc, 0)
                    # norm for (qc,0) of the last chunk runs as a filler
                    # inside (qc,1) so only (qc,1)'s norm sits on the tail
                    if qc == NQ - 1:
                        pre_fill[len(steps)] = (
                            [], [lambda: norm_pair(NQ - 1, 0)])
                    build_unit(qc, 1)

                proj_qk(0)
                proj_v_group(0)
                stv0 = {}
                for tt in (1, 2, 3):
                    for ch in range(2):
                        must.append(lambda t=tt, c=ch, stt=stv0:
                                    proj_v_chunk(t, c, stt))

                def issue_scores(k):
                    if k in pre_fill:
                        m, s = pre_fill[k]
                        must.extend(m)
                        soft.extend(s)
                    if k in drain_before:
                        drain_must()
                    steps[k][0]()

                issue_scores(0)
                for k in range(len(steps)):
                    if k + 1 < len(steps):
                        issue_scores(k + 1)
                    steps[k][1]()
                    if k == 3:
                        nc.scalar.dma_start(wo[:], wo_d[:])
                    run_filler(2)
                while must or soft:
                    run_filler(1)

                norm_pair(NQ - 1, 1)
                for qt in range(4 * (NQ - 1), 4 * NQ):
                    wo_qt(qt, tail=True)

    nc.compile()

    from concourse.bass_interp import get_hw_module
    nc.m = get_hw_module(nc.m)

    _CACHE[key] = nc
    return nc


def _make_mask():
    # mask[p, j] = 1 where (j - p) >= 384; slices of width 512 at offset
    # 384-128*r give the causal mask for a diagonal block at relative
    # position r (k block kb = 4*qc + r vs the 512-wide q chunk qc)
    j = np.arange(896)[None, :]
    p = np.arange(128)[:, None]
    return ((j - p) >= 384).astype(np.float16)


def kernel(x, wq, wk, wv, wo):
    x = np.asarray(x, dtype=np.float32)
    wq = np.asarray(wq, dtype=np.float32)
    wk = np.asarray(wk, dtype=np.float32)
    wv = np.asarray(wv, dtype=np.float32)
    wo = np.asarray(wo, dtype=np.float32)

    from concourse import bass_utils

    nc = _build_program()
    mask = _make_mask()

    def sbuf_w(w):
        # [out=256, in=1024] -> [128, DC, 256] SBUF layout, contiguous DMA
        return np.ascontiguousarray(
            w.T.reshape(DC, 128, F).transpose(1, 0, 2)).astype(np.float16)

    in_maps = []
    for c in range(8):
        b = c // 4
        hg = c % 4
        fs = slice(hg * F, (hg + 1) * F)
        # [NQ, 128, DC, 512]: stripe-major for early projection start
        xT = np.ascontiguousarray(
            x[b].T.reshape(DC, 128, NQ, 512).transpose(2, 1, 0, 3)
        ).astype(np.float16)
        woT = np.ascontiguousarray(
            wo[:, fs].T.reshape(FC, 128, D).transpose(1, 0, 2)
        ).astype(np.float16)
        in_maps.append({
            "xT": xT,
            "wq": sbuf_w(wq[fs, :]),
            "wk": sbuf_w(wk[fs, :]),
            "wv": sbuf_w(wv[fs, :]),
            "wo": woT,
            "mask": mask,
            "ident": np.eye(128, dtype=np.float16),
        })

    res = bass_utils.run_bass_kernel_spmd(nc, in_maps, core_ids=list(range(8)))
    ys = [res.results[c]["y"].astype(np.float32) for c in range(8)]
    out = np.stack([ys[0] + ys[1] + ys[2] + ys[3],
                    ys[4] + ys[5] + ys[6] + ys[7]])
    return out


# revision 25
# speedup vs baseline: 1.1266x; 1.1266x over previous
"""Trainium2 Bass kernel for a 16-head causal MHA layer.

Problem: x:[2,2048,1024] f32, wq/wk/wv/wo:[1024,1024] f32 (Linear-style
[out,in] weights), causal softmax attention with 16 heads of dim 64.

Sharding across the 8 NeuronCores: 2-way data parallel over batch x
4-way tensor parallel over heads.  Core c handles batch c//4 and the 4
heads 4*(c%4) .. 4*(c%4)+3 (feature slice of 256 rows of wq/wk/wv and
256 columns of wo).  Each core produces a partial [2048,1024] output
(its 4 heads' contribution, already projected through its wo slice);
the host sums the 4 partials per batch.

Device dataflow (all matmul inputs fp16, fp32 PSUM accumulation; fp8
was tried and measured 2.1% l2 error -- the softmax does not attenuate
relative error since the attention output shrinks by the same
sqrt(eff_k) factor as the injected score noise -- so everything stays
fp16):
  - x arrives fp16, striped by 512-token chunks, first stripe split
    across both hardware DMA queues so projections start early
  - qT/kT = W @ xT in [feat, token] layout; the reference 1/sqrt(64)
    score scale is folded into the exp activation's free scale slot
  - scoresT[k,q] = kT_h.T-block @ qT_h (64-dim contraction, two heads
    packed onto PE row-halves via tile_position), exp on ACT straight
    out of PSUM, causal mask applied only on diagonal blocks via a
    precomputed 0/1 mask multiply
  - out_unnorm.T | l = (v|1).T-block @ expT accumulated over k blocks
    (the appended ones-column yields the softmax denominator l for free)
  - the whole attention runs as one flat pipeline of (g,half) steps:
    each step's score matmuls are issued one step AHEAD of the exp/AV
    work so the exp engine (the attention pacer at ~2.3us/step vs
    ~1.3us of PE work) never waits on the Tensor queue; projection /
    normalize / output-projection work drains into the leftover PE
    slack from two priority deques (proj chunks must finish before the
    next q-chunk's scores; norm/wo are elastic)
  - 1/l via a DRAM-roundtrip transpose to [128,x] + DVE reciprocal
    (a [1,512] single-lane reciprocal measures 3.3us vs 0.17us for the
    [128,4] layout), broadcast over the dh rows with K=128 ident
    matmuls, two heads packed onto PE column-halves; for the final
    q-chunk the roundtrip latency would sit on the critical tail, so
    l is transposed on-chip with [1,128]->[128,1] PE transposes instead
  - y = outT.T @ woT accumulated over the 256-dim feature slice; the
    two 512-wide output halves share one [128,1024] staging tile and a
    single DMA per 128-token row block; the tail blocks' PSUM->SBUF
    casts are split across Vector and the (by then idle) Scalar engine
"""

import numpy as np

S = 2048          # sequence length (one batch per core)
D = 1024          # model dim
HL = 4            # heads handled per core
DH = 64           # head dim
F = HL * DH       # 256 local features
DC = D // 128     # 8 d_model chunks of 128
FC = F // 128     # 2 feature chunks of 128
NT = S // 128     # 16 token tiles
NQ = S // 512     # 4 query chunks of 512

_CACHE = {}


def _build_program(dbg=False):
    key = ("nc", dbg)
    if key in _CACHE:
        return _CACHE[key]

    import collections

    import concourse.bacc as bacc
    import concourse.bass as bass
    import concourse.mybir as mybir
    import concourse.tile as tile

    f16 = mybir.dt.float16
    f32 = mybir.dt.float32
    Exp = mybir.ActivationFunctionType.Exp

    nc = bacc.Bacc("TRN2", target_bir_lowering=False, debug=False)

    # x striped by 512-token chunks: xT_d[t5][p, dc, j] = x[t5*512+j, dc*128+p]
    xT_d = nc.dram_tensor("xT", [NQ, 128, DC, 512], f16, kind="ExternalInput")
    wq_d = nc.dram_tensor("wq", [128, DC, F], f16, kind="ExternalInput")
    wk_d = nc.dram_tensor("wk", [128, DC, F], f16, kind="ExternalInput")
    wv_d = nc.dram_tensor("wv", [128, DC, F], f16, kind="ExternalInput")
    wo_d = nc.dram_tensor("wo", [128, FC, D], f16, kind="ExternalInput")
    mask_d = nc.dram_tensor("mask", [128, 896], f16, kind="ExternalInput")
    ident_d = nc.dram_tensor("ident", [128, 128], f16, kind="ExternalInput")
    y_d = nc.dram_tensor("y", [S, D], f16, kind="ExternalOutput")

    with tile.TileContext(nc) as tc:
        with tc.tile_pool(name="const", bufs=1) as cpool, \
             tc.tile_pool(name="dscr", bufs=1,
                          space=bass.MemorySpace.DRAM) as dpool:
            l_dram = dpool.tile([HL * S], f32)
            xT = cpool.tile([128, NQ, DC, 512], f16)
            wq = cpool.tile([128, DC, F], f16)
            wk = cpool.tile([128, DC, F], f16)
            wv = cpool.tile([128, DC, F], f16)
            wo = cpool.tile([128, FC, D], f16)
            mask = cpool.tile([128, 896], f16)
            ident = cpool.tile([128, 128], f16)
            qT = cpool.tile([128, FC, S], f16)
            kT = cpool.tile([128, FC, S], f16)
            v = cpool.tile([128, NT, HL, DH + 1], f16)
            outT = cpool.tile([128, FC, S], f16)
            l_row = cpool.tile([1, HL * S], f32)
            lT = cpool.tile([128, HL * NT], f32)
            recipT16 = cpool.tile([128, HL * NT], f16)
            ones1 = cpool.tile([1, 1], f32)

            # loads: the DMA rings round-robin bandwidth across ALL
            # queued transfers, so anything queued early steals from the
            # critical path.  Only the first-needed 3MB goes on the two
            # hardware rings up front (sync: x stripes 0-1, scalar: wq+wk);
            # wv/mask/ident ride the gpsimd software-DGE path, and the
            # stragglers (x stripes 2-3, wo) are emitted later at natural
            # staging points so their transfers start late.
            nc.sync.dma_start(xT[:, 0], xT_d[0])
            nc.scalar.dma_start(wq[:], wq_d[:])
            nc.gpsimd.dma_start(xT[:, 1], xT_d[1])
            nc.gpsimd.dma_start(wv[:], wv_d[:])
            nc.gpsimd.dma_start(mask[:], mask_d[:])
            nc.gpsimd.dma_start(ident[:], ident_d[:])

            # constants / ones columns for the softmax-denominator trick
            nc.gpsimd.memset(v[:], 1.0)
            nc.gpsimd.memset(ones1[:], 1.0)

            with tc.tile_pool(name="sc_ps", bufs=2,
                              space=bass.MemorySpace.PSUM) as scp, \
                 tc.tile_pool(name="av_ps", bufs=2,
                              space=bass.MemorySpace.PSUM) as avp, \
                 tc.tile_pool(name="ybc_ps", bufs=2,
                              space=bass.MemorySpace.PSUM) as ybcp, \
                 tc.tile_pool(name="p_sb", bufs=6) as ppool, \
                 tc.tile_pool(name="y_sb", bufs=3) as ysb_pool:

                # HAM warmup: dummy matmuls during the input-load window so
                # the PE clock-gate is at 8/8 when real work arrives; also
                # pre-trigger the exp ACT table load off the critical path.
                warm = ppool.tile([128, 128], f16, tag="warm", bufs=1)
                warm2 = ppool.tile([128, 128], f16, tag="warm2", bufs=1)
                nc.vector.memset(warm[:], 1.0)
                nc.scalar.dma_start(wk[:], wk_d[:])
                nc.scalar.activation(warm2[:, 0:1], warm[:, 0:1], Exp)
                wps = ybcp.tile([128, 512], f32, tag="ybc", name="warm_ps")
                for _ in range(8):
                    nc.tensor.matmul(
                        wps[:], warm[:],
                        warm[:, 0:1].to_broadcast((128, 512)),
                        start=True, stop=True)

                # quarter-size projection chunks keep the filler
                # granularity near ~0.4us so the per-step drain slots pack
                # the PE slack left by the exp-paced attention pipeline
                NCH = 4

                def proj_qk_chunk(w_sb, dstT, fc, t5, ch, state):
                    if ch == 0:
                        state[fc] = ybcp.tile([128, 512], f32, tag="ybc",
                                              name=f"ps_{t5}_{fc}")
                    ps = state[fc]
                    per = DC // NCH
                    for dc in range(per * ch, per * ch + per):
                        nc.tensor.matmul(
                            ps[:],
                            w_sb[:, dc, fc * 128:(fc + 1) * 128],
                            xT[:, t5, dc, :],
                            start=(dc == 0), stop=(dc == DC - 1))
                    if ch == NCH - 1:
                        nc.vector.tensor_copy(
                            dstT[:, fc, t5 * 512:(t5 + 1) * 512], ps[:])

                def proj_qk(t5):
                    st = {}
                    for w_sb, dstT in ((wq, qT), (wk, kT)):
                        for fc in range(FC):
                            for ch in range(NCH):
                                proj_qk_chunk(w_sb, dstT, fc, t5, ch, st)

                def proj_v_chunk(tt, ch, state):
                    t5, r = divmod(tt, 4)
                    if ch == 0:
                        state[tt] = ybcp.tile([128, F], f32, tag="ybc",
                                              name=f"psv_{tt}")
                    psv = state[tt]
                    for dc in range(4 * ch, 4 * ch + 4):
                        nc.tensor.matmul(
                            psv[:],
                            xT[:, t5, dc, r * 128:(r + 1) * 128],
                            wv[:, dc, :],
                            start=(dc == 0), stop=(dc == DC - 1))
                    if ch == 1:
                        nc.vector.tensor_copy(
                            v[:, tt, :, 0:DH],
                            psv.rearrange("p (h d) -> p h d", h=HL))

                def proj_v_group(tt):
                    st = {}
                    proj_v_chunk(tt, 0, st)
                    proj_v_chunk(tt, 1, st)

                # two priority classes: "must" fillers gate the next
                # q-chunk's scores (projections); "soft" are elastic
                must = collections.deque()
                soft = collections.deque()

                def run_filler(n):
                    for _ in range(n):
                        if must:
                            must.popleft()()
                        elif soft:
                            soft.popleft()()

                def drain_must():
                    while must:
                        must.popleft()()

                def norm_pair(qc, hc):
                    # 1/l on the [q-partition] transposed copy (128 DVE
                    # lanes), broadcast over the dh rows with K=128 ident
                    # matmuls, two heads packed onto PE column halves,
                    # then one tensor_mul normalizes the [128,512] chunk
                    sl = slice(qc * 512, (qc + 1) * 512)
                    if qc == NQ - 1:
                        # on-chip l transpose: [1,128] -> [128,1] PE
                        # transposes into PSUM; skips the DRAM roundtrip
                        # latency that would sit on the critical tail
                        ltp = ybcp.tile([128, 8], f32, tag="ybc",
                                        name=f"ltp_{hc}")
                        for hp2 in range(2):
                            h = hc * 2 + hp2
                            for t4 in range(4):
                                seg = slice(h * S + qc * 512 + t4 * 128,
                                            h * S + qc * 512 + (t4 + 1) * 128)
                                nc.tensor.transpose(
                                    ltp[:, 4 * hp2 + t4:4 * hp2 + t4 + 1],
                                    l_row[0:1, seg], ones1[:])
                        with nc.allow_low_precision(
                                reason="fp16 1/l; l>=1 so ~5e-4 relative"):
                            for hp2 in range(2):
                                h = hc * 2 + hp2
                                cols = slice(h * NT + 4 * qc,
                                             h * NT + 4 * qc + 4)
                                nc.vector.reciprocal(
                                    recipT16[:, cols],
                                    ltp[:, 4 * hp2:4 * hp2 + 4])
                    else:
                        with nc.allow_low_precision(
                                reason="fp16 1/l; l>=1 so ~5e-4 relative"):
                            for hp2 in range(2):
                                h = hc * 2 + hp2
                                cols = slice(h * NT + 4 * qc,
                                             h * NT + 4 * qc + 4)
                                nc.vector.reciprocal(recipT16[:, cols],
                                                     lT[:, cols])
                    bc = ybcp.tile([128, 512], f32, tag="ybc",
                                   name=f"bc_{hc}_{qc}")
                    for hp2 in range(2):
                        for t4 in range(4):
                            col = (hc * 2 + hp2) * NT + 4 * qc + t4
                            nc.tensor.matmul(
                                bc[hp2 * 64:(hp2 + 1) * 64,
                                   t4 * 128:(t4 + 1) * 128],
                                recipT16[:, col:col + 1]
                                .to_broadcast((128, DH)),
                                ident[:],
                                start=True, stop=True,
                                tile_position=(0, hp2 * 64))
                    nc.vector.tensor_mul(
                        outT[:, hc, sl], outT[:, hc, sl], bc[:])

                def wo_qt(qt, tail=False):
                    ysb = ysb_pool.tile([128, 1024], f16, tag="ysb",
                                        name=f"ysb_{qt}")
                    for oc in range(2):
                        yps = ybcp.tile([128, 512], f32, tag="ybc",
                                        name=f"yps_{qt}_{oc}")
                        for fc in range(FC):
                            nc.tensor.matmul(
                                yps[:],
                                outT[:, fc, qt * 128:(qt + 1) * 128],
                                wo[:, fc, oc * 512:(oc + 1) * 512],
                                start=(fc == 0), stop=(fc == FC - 1))
                        dst = ysb[:, oc * 512:(oc + 1) * 512]
                        if tail and oc == 0:
                            # scalar is idle once the exps are done; split
                            # the tail casts so Vector isn't the pacer
                            nc.scalar.copy(dst, yps[:])
                        else:
                            nc.vector.tensor_copy(dst, yps[:])
                    nc.sync.dma_start(
                        y_d[qt * 128:(qt + 1) * 128, :], ysb[:])

                # ---- attention step pipeline ---------------------------
                # one step = one 128-key block for BOTH heads of the pair:
                # 2 score matmuls (PE row-halves) into disjoint column
                # halves of ONE [128,1024] PSUM tile, one exp ACT over a
                # [128,2,w] view, mask on diagonal, 2 AV matmuls.  Scores
                # are issued one step AHEAD of the exp/AV work, and the
                # 2-deep sc pool holds exactly the in-flight step + the
                # lookahead step, so the exp engine (the attention pacer)
                # never waits on the Tensor queue.
                steps = []
                pre_fill = {}     # step idx -> (must list, soft list)
                drain_before = set()

                def build_unit(qc, hc):
                    state = {}

                    def ensure_avs():
                        if 'avs' not in state:
                            state['avs'] = [
                                avp.tile([DH + 1, 512], f32, tag="av",
                                         name=f"av_{hc}_{qc}_{hp2}")
                                for hp2 in range(2)]
                        return state['avs']

                    def finalize_early():
                        # columns [0:384] are final once kb = n_kb-2 is
                        # accumulated (the last, r=3 block only writes
                        # [384:512]); copy them under the last AV
                        avs = state['avs']
                        for hp2 in range(2):
                            h = hc * 2 + hp2
                            nc.vector.tensor_copy(
                                outT[hp2 * 64:hp2 * 64 + 64, hc,
                                     qc * 512:qc * 512 + 384],
                                avs[hp2][0:DH, 0:384])
                            seg = slice(h * S + qc * 512,
                                        h * S + qc * 512 + 384)
                            nc.vector.tensor_copy(
                                l_row[0:1, seg], avs[hp2][DH:DH + 1, 0:384])

                    def finalize():
                        # one small staging read releases the avs PSUM pair
                        # (~0.26us after its last AV); the outT/l fan-out
                        # copies run off the release path
                        avs = state['avs']
                        for hp2 in range(2):
                            h = hc * 2 + hp2
                            stg = ppool.tile([DH + 1, 128], f32,
                                             tag="stg", bufs=2,
                                             name=f"stg_{hc}_{qc}_{hp2}")
                            nc.vector.tensor_copy(stg[:],
                                                  avs[hp2][:, 384:512])
                            nc.vector.tensor_copy(
                                outT[hp2 * 64:hp2 * 64 + 64, hc,
                                     qc * 512 + 384:(qc + 1) * 512],
                                stg[0:DH, :])
                            seg = slice(h * S + qc * 512 + 384,
                                        h * S + (qc + 1) * 512)
                            nc.vector.tensor_copy(
                                l_row[0:1, seg], stg[DH:DH + 1, :])
                            if qc < NQ - 1:
                                seg = slice(h * S + qc * 512,
                                            h * S + (qc + 1) * 512)
                                nc.sync.dma_start(l_dram[seg],
                                                  l_row[0:1, seg])
                                nc.sync.dma_start(
                                    lT[:, h * NT + 4 * qc:
                                       h * NT + 4 * qc + 4],
                                    l_dram[seg]
                                    .rearrange("(t p) -> p t", p=128))

                    n_kb = 4 * (qc + 1)
                    for kb in range(n_kb):
                        r = kb - 4 * qc
                        if r >= 0:
                            qo, w = 128 * r, 512 - 128 * r
                        else:
                            qo, w = 0, 512

                        def scores_fn(kb=kb, qo=qo, w=w):
                            sc = scp.tile([128, 1024], f32, tag="sc",
                                          name=f"sc_{hc}_{qc}_{kb}")
                            for hp2 in range(2):
                                hp = hp2 * 64
                                nc.tensor.matmul(
                                    sc[:, hp2 * 512 + qo:
                                       hp2 * 512 + qo + w],
                                    kT[hp:hp + 64, hc,
                                       kb * 128:(kb + 1) * 128],
                                    qT[hp:hp + 64, hc,
                                       qc * 512 + qo:(qc + 1) * 512],
                                    start=True, stop=True,
                                    tile_position=(hp, 0))
                            state[kb] = sc

                        def rest_fn(kb=kb, qo=qo, w=w, diag=(r >= 0),
                                    last=(kb == n_kb - 1)):
                            sc = state.pop(kb)
                            avs = ensure_avs()
                            p_sb = ppool.tile([128, 1024], f16, tag="p",
                                              name=f"p_{hc}_{qc}_{kb}")
                            sc2 = sc.rearrange("p (h w) -> p h w", h=2)
                            p2 = p_sb.rearrange("p (h w) -> p h w", h=2)
                            # the reference 1/sqrt(64) score scale
                            nc.scalar.activation(
                                p2[:, :, qo:qo + w],
                                sc2[:, :, qo:qo + w], Exp,
                                scale=0.125)
                            if diag:
                                # only the first 128 columns of a clipped
                                # block straddle the diagonal
                                for hp2 in range(2):
                                    o = hp2 * 512 + qo
                                    nc.vector.tensor_mul(
                                        p_sb[:, o:o + 128],
                                        p_sb[:, o:o + 128],
                                        mask[:, 384:512])
                            for hp2 in range(2):
                                h = hc * 2 + hp2
                                nc.tensor.matmul(
                                    avs[hp2][:, qo:512],
                                    v[:, kb, h, :],
                                    p_sb[:, hp2 * 512 + qo:
                                         hp2 * 512 + qo + w],
                                    start=(kb == 0),
                                    stop=(kb == n_kb - 1))
                            if kb == n_kb - 2:
                                finalize_early()
                            if last:
                                finalize()

                        steps.append((scores_fn, rest_fn))

                for qc in range(NQ):
                    idx = len(steps)
                    m, s = [], []
                    if qc + 1 < NQ:
                        st = {}
                        for w_sb, dstT in ((wq, qT), (wk, kT)):
                            for fc in range(FC):
                                for ch in range(NCH):
                                    m.append(
                                        lambda w=w_sb, d=dstT, f=fc,
                                        t=qc + 1, c=ch, stt=st:
                                        proj_qk_chunk(w, d, f, t, c, stt))
                        stv = {}
                        for tt in range(4 * (qc + 1), 4 * (qc + 2)):
                            for ch in range(2):
                                m.append(lambda t=tt, c=ch, stt=stv:
                                         proj_v_chunk(t, c, stt))
                    if qc >= 1:
                        for hcx in range(FC):
                            s.append(
                                lambda q=qc - 1, c=hcx: norm_pair(q, c))
                        for qt in range(4 * (qc - 1), 4 * qc):
                            s.append(lambda a=qt: wo_qt(a))
                    pre_fill[idx] = (m, s)
                    if qc > 0:
                        drain_before.add(idx)
                    build_unit(qc, 0)
                    # norm for (qc,0) of the last chunk runs as a filler
                    # inside (qc,1) so only (qc,1)'s norm sits on the tail
                    if qc == NQ - 1:
                        pre_fill[len(steps)] = (
                            [], [lambda: norm_pair(NQ - 1, 0)])
                    build_unit(qc, 1)

                proj_qk(0)
                proj_v_group(0)
                stv0 = {}
                for tt in (1, 2, 3):
                    for ch in range(2):
                        must.append(lambda t=tt, c=ch, stt=stv0:
                                    proj_v_chunk(t, c, stt))

                def issue_scores(k):
                    if k in pre_fill:
                        m, s = pre_fill[k]
                        must.extend(m)
                        soft.extend(s)
                    if k in drain_before:
                        drain_must()
                    steps[k][0]()

                issue_scores(0)
                for k in range(len(steps)):
                    if k + 1 < len(steps):
                        issue_scores(k + 1)
                    steps[k][1]()
                    if k == 4:
                        # stage the straggler loads behind real deps: a
                        # 1-element copy from qT (written by proj(1)) into
                        # each DMA's dst tile makes the scheduler hold the
                        # transfer until the early loads have drained the
                        # rings (they round-robin bandwidth otherwise)
                        for dst, dd, eng in ((xT[0:1, 2, 0, 0:1], None,
                                              None),
                                             (xT[0:1, 3, 0, 0:1], None,
                                              None),
                                             (wo[0:1, 0, 0:1], None,
                                              None)):
                            nc.vector.tensor_copy(dst,
                                                  qT[0:1, 0, 512:513])
                        nc.sync.dma_start(xT[:, 2], xT_d[2])
                        nc.sync.dma_start(xT[:, 3], xT_d[3])
                        nc.scalar.dma_start(wo[:], wo_d[:])
                    run_filler(2)
                while must or soft:
                    run_filler(1)

                norm_pair(NQ - 1, 1)
                for qt in range(4 * (NQ - 1), 4 * NQ):
                    wo_qt(qt, tail=True)

    nc.compile()

    from concourse.bass_interp import get_hw_module
    nc.m = get_hw_module(nc.m)

    _CACHE[key] = nc
    return nc


def _make_mask():
    # mask[p, j] = 1 where (j - p) >= 384; slices of width 512 at offset
    # 384-128*r give the causal mask for a diagonal block at relative
    # position r (k block kb = 4*qc + r vs the 512-wide q chunk qc)
    j = np.arange(896)[None, :]
    p = np.arange(128)[:, None]
    return ((j - p) >= 384).astype(np.float16)


def kernel(x, wq, wk, wv, wo):
    x = np.asarray(x, dtype=np.float32)
    wq = np.asarray(wq, dtype=np.float32)
    wk = np.asarray(wk, dtype=np.float32)
    wv = np.asarray(wv, dtype=np.float32)
    wo = np.asarray(wo, dtype=np.float32)

    from concourse import bass_utils

    nc = _build_program()
    mask = _make_mask()

    def sbuf_w(w):
        # [out=256, in=1024] -> [128, DC, 256] SBUF layout, contiguous DMA
        return np.ascontiguousarray(
            w.T.reshape(DC, 128, F).transpose(1, 0, 2)).astype(np.float16)

    in_maps = []
    for c in range(8):
        b = c // 4
        hg = c % 4
        fs = slice(hg * F, (hg + 1) * F)
        # [NQ, 128, DC, 512]: stripe-major for early projection start
        xT = np.ascontiguousarray(
            x[b].T.reshape(DC, 128, NQ, 512).transpose(2, 1, 0, 3)
        ).astype(np.float16)
        woT = np.ascontiguousarray(
            wo[:, fs].T.reshape(FC, 128, D).transpose(1, 0, 2)
        ).astype(np.float16)
        in_maps.append({
            "xT": xT,
            "wq": sbuf_w(wq[fs, :]),
            "wk": sbuf_w(wk[fs, :]),
            "wv": sbuf_w(wv[fs, :]),
            "wo": woT,
            "mask": mask,
            "ident": np.eye(128, dtype=np.float16),
        })

    res = bass_utils.run_bass_kernel_spmd(nc, in_maps, core_ids=list(range(8)))
    ys = [res.results[c]["y"].astype(np.float32) for c in range(8)]
    out = np.stack([ys[0] + ys[1] + ys[2] + ys[3],
                    ys[4] + ys[5] + ys[6] + ys[7]])
    return out


# revision 26
# speedup vs baseline: 1.1299x; 1.0029x over previous
"""Trainium2 Bass kernel for a 16-head causal MHA layer.

Problem: x:[2,2048,1024] f32, wq/wk/wv/wo:[1024,1024] f32 (Linear-style
[out,in] weights), causal softmax attention with 16 heads of dim 64.

Sharding across the 8 NeuronCores: 2-way data parallel over batch x
4-way tensor parallel over heads.  Core c handles batch c//4 and the 4
heads 4*(c%4) .. 4*(c%4)+3 (feature slice of 256 rows of wq/wk/wv and
256 columns of wo).  Each core produces a partial [2048,1024] output
(its 4 heads' contribution, already projected through its wo slice);
the host sums the 4 partials per batch.

Device dataflow (all matmul inputs fp16, fp32 PSUM accumulation; fp8
was tried and measured 2.1% l2 error -- the softmax does not attenuate
relative error since the attention output shrinks by the same
sqrt(eff_k) factor as the injected score noise -- so everything stays
fp16):
  - x arrives fp16, striped by 512-token chunks, first stripe split
    across both hardware DMA queues so projections start early
  - qT/kT = W @ xT in [feat, token] layout; the reference 1/sqrt(64)
    score scale is folded into the exp activation's free scale slot
  - scoresT[k,q] = kT_h.T-block @ qT_h (64-dim contraction, two heads
    packed onto PE row-halves via tile_position), exp on ACT straight
    out of PSUM, causal mask applied only on diagonal blocks via a
    precomputed 0/1 mask multiply
  - out_unnorm.T | l = (v|1).T-block @ expT accumulated over k blocks
    (the appended ones-column yields the softmax denominator l for free)
  - the whole attention runs as one flat pipeline of (g,half) steps:
    each step's score matmuls are issued one step AHEAD of the exp/AV
    work so the exp engine (the attention pacer at ~2.3us/step vs
    ~1.3us of PE work) never waits on the Tensor queue; projection /
    normalize / output-projection work drains into the leftover PE
    slack from two priority deques (proj chunks must finish before the
    next q-chunk's scores; norm/wo are elastic)
  - 1/l via a DRAM-roundtrip transpose to [128,x] + DVE reciprocal
    (a [1,512] single-lane reciprocal measures 3.3us vs 0.17us for the
    [128,4] layout), broadcast over the dh rows with K=128 ident
    matmuls, two heads packed onto PE column-halves; for the final
    q-chunk the roundtrip latency would sit on the critical tail, so
    l is transposed on-chip with [1,128]->[128,1] PE transposes instead
  - y = outT.T @ woT accumulated over the 256-dim feature slice; the
    two 512-wide output halves share one [128,1024] staging tile and a
    single DMA per 128-token row block; the tail blocks' PSUM->SBUF
    casts are split across Vector and the (by then idle) Scalar engine
"""

import numpy as np

S = 2048          # sequence length (one batch per core)
D = 1024          # model dim
HL = 4            # heads handled per core
DH = 64           # head dim
F = HL * DH       # 256 local features
DC = D // 128     # 8 d_model chunks of 128
FC = F // 128     # 2 feature chunks of 128
NT = S // 128     # 16 token tiles
NQ = S // 512     # 4 query chunks of 512

_CACHE = {}


def _build_program(dbg=False):
    key = ("nc", dbg)
    if key in _CACHE:
        return _CACHE[key]

    import collections

    import concourse.bacc as bacc
    import concourse.bass as bass
    import concourse.mybir as mybir
    import concourse.tile as tile

    f16 = mybir.dt.float16
    f32 = mybir.dt.float32
    Exp = mybir.ActivationFunctionType.Exp

    nc = bacc.Bacc("TRN2", target_bir_lowering=False, debug=False)

    # x striped by 512-token chunks: xT_d[t5][p, dc, j] = x[t5*512+j, dc*128+p]
    xT_d = nc.dram_tensor("xT", [NQ, 128, DC, 512], f16, kind="ExternalInput")
    wq_d = nc.dram_tensor("wq", [128, DC, F], f16, kind="ExternalInput")
    wk_d = nc.dram_tensor("wk", [128, DC, F], f16, kind="ExternalInput")
    wv_d = nc.dram_tensor("wv", [128, DC, F], f16, kind="ExternalInput")
    wo_d = nc.dram_tensor("wo", [128, FC, D], f16, kind="ExternalInput")
    mask_d = nc.dram_tensor("mask", [128, 896], f16, kind="ExternalInput")
    ident_d = nc.dram_tensor("ident", [128, 128], f16, kind="ExternalInput")
    y_d = nc.dram_tensor("y", [S, D], f16, kind="ExternalOutput")

    with tile.TileContext(nc) as tc:
        with tc.tile_pool(name="const", bufs=1) as cpool, \
             tc.tile_pool(name="dscr", bufs=1,
                          space=bass.MemorySpace.DRAM) as dpool:
            l_dram = dpool.tile([HL * S], f32)
            xT = cpool.tile([128, NQ, DC, 512], f16)
            wq = cpool.tile([128, DC, F], f16)
            wk = cpool.tile([128, DC, F], f16)
            wv = cpool.tile([128, DC, F], f16)
            wo = cpool.tile([128, FC, D], f16)
            mask = cpool.tile([128, 896], f16)
            ident = cpool.tile([128, 128], f16)
            qT = cpool.tile([128, FC, S], f16)
            kT = cpool.tile([128, FC, S], f16)
            v = cpool.tile([128, NT, HL, DH + 1], f16)
            outT = cpool.tile([128, FC, S], f16)
            l_row = cpool.tile([1, HL * S], f32)
            lT = cpool.tile([128, HL * NT], f32)
            recipT16 = cpool.tile([128, HL * NT], f16)
            ones1 = cpool.tile([1, 1], f32)

            # loads: the DMA rings round-robin bandwidth across ALL
            # queued transfers, so anything queued early steals from the
            # critical path.  Only the first-needed 3MB goes on the two
            # hardware rings up front (sync: x stripes 0-1, scalar: wq+wk);
            # wv/mask/ident ride the gpsimd software-DGE path, and the
            # stragglers (x stripes 2-3, wo) are emitted later at natural
            # staging points so their transfers start late.
            nc.sync.dma_start(xT[:, 0], xT_d[0])
            nc.scalar.dma_start(wq[:], wq_d[:])
            nc.sync.dma_start(xT[:, 1], xT_d[1])
            nc.gpsimd.dma_start(wv[:], wv_d[:])
            nc.gpsimd.dma_start(mask[:], mask_d[:])
            nc.gpsimd.dma_start(ident[:], ident_d[:])

            # constants / ones columns for the softmax-denominator trick
            nc.gpsimd.memset(v[:], 1.0)
            nc.gpsimd.memset(ones1[:], 1.0)

            with tc.tile_pool(name="sc_ps", bufs=2,
                              space=bass.MemorySpace.PSUM) as scp, \
                 tc.tile_pool(name="av_ps", bufs=2,
                              space=bass.MemorySpace.PSUM) as avp, \
                 tc.tile_pool(name="ybc_ps", bufs=2,
                              space=bass.MemorySpace.PSUM) as ybcp, \
                 tc.tile_pool(name="p_sb", bufs=6) as ppool, \
                 tc.tile_pool(name="y_sb", bufs=3) as ysb_pool:

                # HAM warmup: dummy matmuls during the input-load window so
                # the PE clock-gate is at 8/8 when real work arrives; also
                # pre-trigger the exp ACT table load off the critical path.
                warm = ppool.tile([128, 128], f16, tag="warm", bufs=1)
                warm2 = ppool.tile([128, 128], f16, tag="warm2", bufs=1)
                nc.vector.memset(warm[:], 1.0)
                nc.scalar.dma_start(wk[:], wk_d[:])
                nc.scalar.activation(warm2[:, 0:1], warm[:, 0:1], Exp)
                wps = ybcp.tile([128, 512], f32, tag="ybc", name="warm_ps")
                for _ in range(8):
                    nc.tensor.matmul(
                        wps[:], warm[:],
                        warm[:, 0:1].to_broadcast((128, 512)),
                        start=True, stop=True)

                # quarter-size projection chunks keep the filler
                # granularity near ~0.4us so the per-step drain slots pack
                # the PE slack left by the exp-paced attention pipeline
                NCH = 4

                def proj_qk_chunk(w_sb, dstT, fc, t5, ch, state):
                    if ch == 0:
                        state[fc] = ybcp.tile([128, 512], f32, tag="ybc",
                                              name=f"ps_{t5}_{fc}")
                    ps = state[fc]
                    per = DC // NCH
                    for dc in range(per * ch, per * ch + per):
                        nc.tensor.matmul(
                            ps[:],
                            w_sb[:, dc, fc * 128:(fc + 1) * 128],
                            xT[:, t5, dc, :],
                            start=(dc == 0), stop=(dc == DC - 1))
                    if ch == NCH - 1:
                        nc.vector.tensor_copy(
                            dstT[:, fc, t5 * 512:(t5 + 1) * 512], ps[:])

                def proj_qk(t5):
                    st = {}
                    for w_sb, dstT in ((wq, qT), (wk, kT)):
                        for fc in range(FC):
                            for ch in range(NCH):
                                proj_qk_chunk(w_sb, dstT, fc, t5, ch, st)

                def proj_v_chunk(tt, ch, state):
                    t5, r = divmod(tt, 4)
                    if ch == 0:
                        state[tt] = ybcp.tile([128, F], f32, tag="ybc",
                                              name=f"psv_{tt}")
                    psv = state[tt]
                    for dc in range(4 * ch, 4 * ch + 4):
                        nc.tensor.matmul(
                            psv[:],
                            xT[:, t5, dc, r * 128:(r + 1) * 128],
                            wv[:, dc, :],
                            start=(dc == 0), stop=(dc == DC - 1))
                    if ch == 1:
                        nc.vector.tensor_copy(
                            v[:, tt, :, 0:DH],
                            psv.rearrange("p (h d) -> p h d", h=HL))

                def proj_v_group(tt):
                    st = {}
                    proj_v_chunk(tt, 0, st)
                    proj_v_chunk(tt, 1, st)

                # two priority classes: "must" fillers gate the next
                # q-chunk's scores (projections); "soft" are elastic
                must = collections.deque()
                soft = collections.deque()

                def run_filler(n):
                    for _ in range(n):
                        if must:
                            must.popleft()()
                        elif soft:
                            soft.popleft()()

                def drain_must():
                    while must:
                        must.popleft()()

                def norm_pair(qc, hc):
                    # 1/l on the [q-partition] transposed copy (128 DVE
                    # lanes), broadcast over the dh rows with K=128 ident
                    # matmuls, two heads packed onto PE column halves,
                    # then one tensor_mul normalizes the [128,512] chunk
                    sl = slice(qc * 512, (qc + 1) * 512)
                    if qc == NQ - 1:
                        # on-chip l transpose: [1,128] -> [128,1] PE
                        # transposes into PSUM; skips the DRAM roundtrip
                        # latency that would sit on the critical tail
                        ltp = ybcp.tile([128, 8], f32, tag="ybc",
                                        name=f"ltp_{hc}")
                        for hp2 in range(2):
                            h = hc * 2 + hp2
                            for t4 in range(4):
                                seg = slice(h * S + qc * 512 + t4 * 128,
                                            h * S + qc * 512 + (t4 + 1) * 128)
                                nc.tensor.transpose(
                                    ltp[:, 4 * hp2 + t4:4 * hp2 + t4 + 1],
                                    l_row[0:1, seg], ones1[:])
                        with nc.allow_low_precision(
                                reason="fp16 1/l; l>=1 so ~5e-4 relative"):
                            for hp2 in range(2):
                                h = hc * 2 + hp2
                                cols = slice(h * NT + 4 * qc,
                                             h * NT + 4 * qc + 4)
                                nc.vector.reciprocal(
                                    recipT16[:, cols],
                                    ltp[:, 4 * hp2:4 * hp2 + 4])
                    else:
                        with nc.allow_low_precision(
                                reason="fp16 1/l; l>=1 so ~5e-4 relative"):
                            for hp2 in range(2):
                                h = hc * 2 + hp2
                                cols = slice(h * NT + 4 * qc,
                                             h * NT + 4 * qc + 4)
                                nc.vector.reciprocal(recipT16[:, cols],
                                                     lT[:, cols])
                    bc = ybcp.tile([128, 512], f32, tag="ybc",
                                   name=f"bc_{hc}_{qc}")
                    for hp2 in range(2):
                        for t4 in range(4):
                            col = (hc * 2 + hp2) * NT + 4 * qc + t4
                            nc.tensor.matmul(
                                bc[hp2 * 64:(hp2 + 1) * 64,
                                   t4 * 128:(t4 + 1) * 128],
                                recipT16[:, col:col + 1]
                                .to_broadcast((128, DH)),
                                ident[:],
                                start=True, stop=True,
                                tile_position=(0, hp2 * 64))
                    nc.vector.tensor_mul(
                        outT[:, hc, sl], outT[:, hc, sl], bc[:])

                def wo_qt(qt, tail=False):
                    ysb = ysb_pool.tile([128, 1024], f16, tag="ysb",
                                        name=f"ysb_{qt}")
                    for oc in range(2):
                        yps = ybcp.tile([128, 512], f32, tag="ybc",
                                        name=f"yps_{qt}_{oc}")
                        for fc in range(FC):
                            nc.tensor.matmul(
                                yps[:],
                                outT[:, fc, qt * 128:(qt + 1) * 128],
                                wo[:, fc, oc * 512:(oc + 1) * 512],
                                start=(fc == 0), stop=(fc == FC - 1))
                        dst = ysb[:, oc * 512:(oc + 1) * 512]
                        if tail and oc == 0:
                            # scalar is idle once the exps are done; split
                            # the tail casts so Vector isn't the pacer
                            nc.scalar.copy(dst, yps[:])
                        else:
                            nc.vector.tensor_copy(dst, yps[:])
                    nc.sync.dma_start(
                        y_d[qt * 128:(qt + 1) * 128, :], ysb[:])

                # ---- attention step pipeline ---------------------------
                # one step = one 128-key block for BOTH heads of the pair:
                # 2 score matmuls (PE row-halves) into disjoint column
                # halves of ONE [128,1024] PSUM tile, one exp ACT over a
                # [128,2,w] view, mask on diagonal, 2 AV matmuls.  Scores
                # are issued one step AHEAD of the exp/AV work, and the
                # 2-deep sc pool holds exactly the in-flight step + the
                # lookahead step, so the exp engine (the attention pacer)
                # never waits on the Tensor queue.
                steps = []
                pre_fill = {}     # step idx -> (must list, soft list)
                drain_before = set()

                def build_unit(qc, hc):
                    state = {}

                    def ensure_avs():
                        if 'avs' not in state:
                            state['avs'] = [
                                avp.tile([DH + 1, 512], f32, tag="av",
                                         name=f"av_{hc}_{qc}_{hp2}")
                                for hp2 in range(2)]
                        return state['avs']

                    def finalize_early():
                        # columns [0:384] are final once kb = n_kb-2 is
                        # accumulated (the last, r=3 block only writes
                        # [384:512]); copy them under the last AV
                        avs = state['avs']
                        for hp2 in range(2):
                            h = hc * 2 + hp2
                            nc.vector.tensor_copy(
                                outT[hp2 * 64:hp2 * 64 + 64, hc,
                                     qc * 512:qc * 512 + 384],
                                avs[hp2][0:DH, 0:384])
                            seg = slice(h * S + qc * 512,
                                        h * S + qc * 512 + 384)
                            nc.vector.tensor_copy(
                                l_row[0:1, seg], avs[hp2][DH:DH + 1, 0:384])

                    def finalize():
                        # one small staging read releases the avs PSUM pair
                        # (~0.26us after its last AV); the outT/l fan-out
                        # copies run off the release path
                        avs = state['avs']
                        for hp2 in range(2):
                            h = hc * 2 + hp2
                            stg = ppool.tile([DH + 1, 128], f32,
                                             tag="stg", bufs=2,
                                             name=f"stg_{hc}_{qc}_{hp2}")
                            nc.vector.tensor_copy(stg[:],
                                                  avs[hp2][:, 384:512])
                            nc.vector.tensor_copy(
                                outT[hp2 * 64:hp2 * 64 + 64, hc,
                                     qc * 512 + 384:(qc + 1) * 512],
                                stg[0:DH, :])
                            seg = slice(h * S + qc * 512 + 384,
                                        h * S + (qc + 1) * 512)
                            nc.vector.tensor_copy(
                                l_row[0:1, seg], stg[DH:DH + 1, :])
                            if qc < NQ - 1:
                                seg = slice(h * S + qc * 512,
                                            h * S + (qc + 1) * 512)
                                nc.sync.dma_start(l_dram[seg],
                                                  l_row[0:1, seg])
                                nc.sync.dma_start(
                                    lT[:, h * NT + 4 * qc:
                                       h * NT + 4 * qc + 4],
                                    l_dram[seg]
                                    .rearrange("(t p) -> p t", p=128))

                    n_kb = 4 * (qc + 1)
                    for kb in range(n_kb):
                        r = kb - 4 * qc
                        if r >= 0:
                            qo, w = 128 * r, 512 - 128 * r
                        else:
                            qo, w = 0, 512

                        def scores_fn(kb=kb, qo=qo, w=w):
                            sc = scp.tile([128, 1024], f32, tag="sc",
                                          name=f"sc_{hc}_{qc}_{kb}")
                            for hp2 in range(2):
                                hp = hp2 * 64
                                nc.tensor.matmul(
                                    sc[:, hp2 * 512 + qo:
                                       hp2 * 512 + qo + w],
                                    kT[hp:hp + 64, hc,
                                       kb * 128:(kb + 1) * 128],
                                    qT[hp:hp + 64, hc,
                                       qc * 512 + qo:(qc + 1) * 512],
                                    start=True, stop=True,
                                    tile_position=(hp, 0))
                            state[kb] = sc

                        def rest_fn(kb=kb, qo=qo, w=w, diag=(r >= 0),
                                    last=(kb == n_kb - 1)):
                            sc = state.pop(kb)
                            avs = ensure_avs()
                            p_sb = ppool.tile([128, 1024], f16, tag="p",
                                              name=f"p_{hc}_{qc}_{kb}")
                            sc2 = sc.rearrange("p (h w) -> p h w", h=2)
                            p2 = p_sb.rearrange("p (h w) -> p h w", h=2)
                            # the reference 1/sqrt(64) score scale
                            nc.scalar.activation(
                                p2[:, :, qo:qo + w],
                                sc2[:, :, qo:qo + w], Exp,
                                scale=0.125)
                            if diag:
                                # only the first 128 columns of a clipped
                                # block straddle the diagonal
                                for hp2 in range(2):
                                    o = hp2 * 512 + qo
                                    nc.vector.tensor_mul(
                                        p_sb[:, o:o + 128],
                                        p_sb[:, o:o + 128],
                                        mask[:, 384:512])
                            for hp2 in range(2):
                                h = hc * 2 + hp2
                                nc.tensor.matmul(
                                    avs[hp2][:, qo:512],
                                    v[:, kb, h, :],
                                    p_sb[:, hp2 * 512 + qo:
                                         hp2 * 512 + qo + w],
                                    start=(kb == 0),
                                    stop=(kb == n_kb - 1))
                            if kb == n_kb - 2:
                                finalize_early()
                            if last:
                                finalize()

                        steps.append((scores_fn, rest_fn))

                for qc in range(NQ):
                    idx = len(steps)
                    m, s = [], []
                    if qc + 1 < NQ:
                        st = {}
                        for w_sb, dstT in ((wq, qT), (wk, kT)):
                            for fc in range(FC):
                                for ch in range(NCH):
                                    m.append(
                                        lambda w=w_sb, d=dstT, f=fc,
                                        t=qc + 1, c=ch, stt=st:
                                        proj_qk_chunk(w, d, f, t, c, stt))
                        stv = {}
                        for tt in range(4 * (qc + 1), 4 * (qc + 2)):
                            for ch in range(2):
                                m.append(lambda t=tt, c=ch, stt=stv:
                                         proj_v_chunk(t, c, stt))
                    if qc >= 1:
                        for hcx in range(FC):
                            s.append(
                                lambda q=qc - 1, c=hcx: norm_pair(q, c))
                        for qt in range(4 * (qc - 1), 4 * qc):
                            s.append(lambda a=qt: wo_qt(a))
                    pre_fill[idx] = (m, s)
                    if qc > 0:
                        drain_before.add(idx)
                    build_unit(qc, 0)
                    # norm for (qc,0) of the last chunk runs as a filler
                    # inside (qc,1) so only (qc,1)'s norm sits on the tail
                    if qc == NQ - 1:
                        pre_fill[len(steps)] = (
                            [], [lambda: norm_pair(NQ - 1, 0)])
                    build_unit(qc, 1)

                proj_qk(0)
                proj_v_group(0)
                stv0 = {}
                for tt in (1, 2, 3):
                    for ch in range(2):
                        must.append(lambda t=tt, c=ch, stt=stv0:
                                    proj_v_chunk(t, c, stt))

                def issue_scores(k):
                    if k in pre_fill:
                        m, s = pre_fill[k]
                        must.extend(m)
                        soft.extend(s)
                    if k in drain_before:
                        drain_must()
                    steps[k][0]()

                issue_scores(0)
                for k in range(len(steps)):
                    if k + 1 < len(steps):
                        issue_scores(k + 1)
                    steps[k][1]()
                    if k == 12:
                        # stage the straggler loads behind real deps: a
                        # 1-element copy from qT stripe 1 (emitted by the
                        # qc=1 boundary drain, before this point) into each
                        # DMA's dst tile makes the scheduler hold the
                        # transfer until the early loads have drained the
                        # rings (they round-robin bandwidth otherwise)
                        for dst in (xT[0:1, 2, 0, 0:1],
                                    xT[0:1, 3, 0, 0:1],
                                    wo[0:1, 0, 0:1]):
                            nc.vector.tensor_copy(dst,
                                                  qT[0:1, 0, 512:513])
                        nc.sync.dma_start(xT[:, 2], xT_d[2])
                        nc.sync.dma_start(xT[:, 3], xT_d[3])
                        nc.scalar.dma_start(wo[:], wo_d[:])
                    run_filler(3 if k < 8 else 2)
                while must or soft:
                    run_filler(1)

                norm_pair(NQ - 1, 1)
                for qt in range(4 * (NQ - 1), 4 * NQ):
                    wo_qt(qt, tail=True)

    nc.compile()

    from concourse.bass_interp import get_hw_module
    nc.m = get_hw_module(nc.m)

    _CACHE[key] = nc
    return nc


def _make_mask():
    # mask[p, j] = 1 where (j - p) >= 384; slices of width 512 at offset
    # 384-128*r give the causal mask for a diagonal block at relative
    # position r (k block kb = 4*qc + r vs the 512-wide q chunk qc)
    j = np.arange(896)[None, :]
    p = np.arange(128)[:, None]
    return ((j - p) >= 384).astype(np.float16)


def kernel(x, wq, wk, wv, wo):
    x = np.asarray(x, dtype=np.float32)
    wq = np.asarray(wq, dtype=np.float32)
    wk = np.asarray(wk, dtype=np.float32)
    wv = np.asarray(wv, dtype=np.float32)
    wo = np.asarray(wo, dtype=np.float32)

    from concourse import bass_utils

    nc = _build_program()
    mask = _make_mask()

    def sbuf_w(w):
        # [out=256, in=1024] -> [128, DC, 256] SBUF layout, contiguous DMA
        return np.ascontiguousarray(
            w.T.reshape(DC, 128, F).transpose(1, 0, 2)).astype(np.float16)

    in_maps = []
    for c in range(8):
        b = c // 4
        hg = c % 4
        fs = slice(hg * F, (hg + 1) * F)
        # [NQ, 128, DC, 512]: stripe-major for early projection start
        xT = np.ascontiguousarray(
            x[b].T.reshape(DC, 128, NQ, 512).transpose(2, 1, 0, 3)
        ).astype(np.float16)
        woT = np.ascontiguousarray(
            wo[:, fs].T.reshape(FC, 128, D).transpose(1, 0, 2)
        ).astype(np.float16)
        in_maps.append({
            "xT": xT,
            "wq": sbuf_w(wq[fs, :]),
            "wk": sbuf_w(wk[fs, :]),
            "wv": sbuf_w(wv[fs, :]),
            "wo": woT,
            "mask": mask,
            "ident": np.eye(128, dtype=np.float16),
        })

    res = bass_utils.run_bass_kernel_spmd(nc, in_maps, core_ids=list(range(8)))
    ys = [res.results[c]["y"].astype(np.float32) for c in range(8)]
    out = np.stack([ys[0] + ys[1] + ys[2] + ys[3],
                    ys[4] + ys[5] + ys[6] + ys[7]])
    return out


# revision 27
# speedup vs baseline: 1.1645x; 1.0306x over previous
"""Trainium2 Bass kernel for a 16-head causal MHA layer.

Problem: x:[2,2048,1024] f32, wq/wk/wv/wo:[1024,1024] f32 (Linear-style
[out,in] weights), causal softmax attention with 16 heads of dim 64.

Sharding across the 8 NeuronCores: 2-way data parallel over batch x
4-way tensor parallel over heads.  Core c handles batch c//4 and the 4
heads 4*(c%4) .. 4*(c%4)+3 (feature slice of 256 rows of wq/wk/wv and
256 columns of wo).  Each core produces a partial [2048,1024] output
(its 4 heads' contribution, already projected through its wo slice);
the host sums the 4 partials per batch.

Device dataflow (all matmul inputs fp16, fp32 PSUM accumulation; fp8
was tried and measured 2.1% l2 error -- the softmax does not attenuate
relative error since the attention output shrinks by the same
sqrt(eff_k) factor as the injected score noise -- so everything stays
fp16):
  - x arrives fp16, striped by 512-token chunks, first stripe split
    across both hardware DMA queues so projections start early
  - qT/kT = W @ xT in [feat, token] layout; the reference 1/sqrt(64)
    score scale is folded into the exp activation's free scale slot
  - scoresT[k,q] = kT_h.T-block @ qT_h (64-dim contraction, two heads
    packed onto PE row-halves via tile_position), exp on ACT straight
    out of PSUM, causal mask applied only on diagonal blocks via a
    precomputed 0/1 mask multiply
  - out_unnorm.T | l = (v|1).T-block @ expT accumulated over k blocks
    (the appended ones-column yields the softmax denominator l for free)
  - the whole attention runs as one flat pipeline of (g,half) steps:
    each step's score matmuls are issued one step AHEAD of the exp/AV
    work so the exp engine (the attention pacer at ~2.3us/step vs
    ~1.3us of PE work) never waits on the Tensor queue; projection /
    normalize / output-projection work drains into the leftover PE
    slack from two priority deques (proj chunks must finish before the
    next q-chunk's scores; norm/wo are elastic)
  - 1/l via a DRAM-roundtrip transpose to [128,x] + DVE reciprocal
    (a [1,512] single-lane reciprocal measures 3.3us vs 0.17us for the
    [128,4] layout), broadcast over the dh rows with K=128 ident
    matmuls, two heads packed onto PE column-halves; for the final
    q-chunk the roundtrip latency would sit on the critical tail, so
    l is transposed on-chip with [1,128]->[128,1] PE transposes instead
  - y = outT.T @ woT accumulated over the 256-dim feature slice; the
    two 512-wide output halves share one [128,1024] staging tile and a
    single DMA per 128-token row block; the tail blocks' PSUM->SBUF
    casts are split across Vector and the (by then idle) Scalar engine
"""

import numpy as np

S = 2048          # sequence length (one batch per core)
D = 1024          # model dim
HL = 4            # heads handled per core
DH = 64           # head dim
F = HL * DH       # 256 local features
DC = D // 128     # 8 d_model chunks of 128
FC = F // 128     # 2 feature chunks of 128
NT = S // 128     # 16 token tiles
NQ = S // 512     # 4 query chunks of 512

_CACHE = {}


def _build_program(dbg=False):
    key = ("nc", dbg)
    if key in _CACHE:
        return _CACHE[key]

    import collections

    import concourse.bacc as bacc
    import concourse.bass as bass
    import concourse.mybir as mybir
    import concourse.tile as tile

    f16 = mybir.dt.float16
    f32 = mybir.dt.float32
    Exp = mybir.ActivationFunctionType.Exp

    nc = bacc.Bacc("TRN2", target_bir_lowering=False, debug=False)

    # x striped by 512-token chunks: xT_d[t5][p, dc, j] = x[t5*512+j, dc*128+p]
    xT_d = nc.dram_tensor("xT", [NQ, 128, DC, 512], f16, kind="ExternalInput")
    wq_d = nc.dram_tensor("wq", [128, DC, F], f16, kind="ExternalInput")
    wk_d = nc.dram_tensor("wk", [128, DC, F], f16, kind="ExternalInput")
    wv_d = nc.dram_tensor("wv", [128, DC, F], f16, kind="ExternalInput")
    wo_d = nc.dram_tensor("wo", [128, FC, D], f16, kind="ExternalInput")
    mask_d = nc.dram_tensor("mask", [128, 896], f16, kind="ExternalInput")
    ident_d = nc.dram_tensor("ident", [128, 128], f16, kind="ExternalInput")
    y_d = nc.dram_tensor("y", [S, D], f16, kind="ExternalOutput")

    with tile.TileContext(nc) as tc:
        with tc.tile_pool(name="const", bufs=1) as cpool, \
             tc.tile_pool(name="dscr", bufs=1,
                          space=bass.MemorySpace.DRAM) as dpool:
            l_dram = dpool.tile([HL * S], f32)
            xT = cpool.tile([128, NQ, DC, 512], f16)
            wq = cpool.tile([128, DC, F], f16)
            wk = cpool.tile([128, DC, F], f16)
            wv = cpool.tile([128, DC, F], f16)
            wo = cpool.tile([128, FC, D], f16)
            mask = cpool.tile([128, 896], f16)
            ident = cpool.tile([128, 128], f16)
            qT = cpool.tile([128, FC, S], f16)
            kT = cpool.tile([128, FC, S], f16)
            v = cpool.tile([128, NT, HL, DH + 1], f16)
            outT = cpool.tile([128, FC, S], f16)
            l_row = cpool.tile([1, HL * S], f32)
            lT = cpool.tile([128, HL * NT], f32)
            recipT16 = cpool.tile([128, HL * NT], f16)
            ones1 = cpool.tile([1, 1], f32)

            # loads: the DMA rings round-robin bandwidth across ALL
            # queued transfers, so anything queued early steals from the
            # critical path.  Only the first-needed 3MB goes on the two
            # hardware rings up front (sync: x stripes 0-1, scalar: wq+wk);
            # wv/mask/ident ride the gpsimd software-DGE path, and the
            # stragglers (x stripes 2-3, wo) are emitted later at natural
            # staging points so their transfers start late.
            nc.sync.dma_start(xT[:, 0], xT_d[0])
            nc.scalar.dma_start(wq[:], wq_d[:])
            # chain the second wave behind the first via 1-element trigger
            # copies (real RAW->WAW deps): the rings round-robin bandwidth
            # across queued transfers, so concurrency delays the critical
            # first items; chaining restores FIFO behavior
            nc.vector.tensor_copy(xT[0:1, 1, 0, 0:1], xT[0:1, 0, 0, 0:1])
            nc.sync.dma_start(xT[:, 1], xT_d[1])
            nc.vector.tensor_copy(wk[0:1, 0, 0:1], wq[0:1, 0, 0:1])
            nc.scalar.dma_start(wk[:], wk_d[:])
            nc.gpsimd.dma_start(wv[:], wv_d[:])
            nc.gpsimd.dma_start(mask[:], mask_d[:])
            nc.gpsimd.dma_start(ident[:], ident_d[:])

            # constants / ones columns for the softmax-denominator trick
            nc.gpsimd.memset(v[:], 1.0)
            nc.gpsimd.memset(ones1[:], 1.0)

            with tc.tile_pool(name="sc_ps", bufs=2,
                              space=bass.MemorySpace.PSUM) as scp, \
                 tc.tile_pool(name="av_ps", bufs=2,
                              space=bass.MemorySpace.PSUM) as avp, \
                 tc.tile_pool(name="ybc_ps", bufs=2,
                              space=bass.MemorySpace.PSUM) as ybcp, \
                 tc.tile_pool(name="p_sb", bufs=6) as ppool, \
                 tc.tile_pool(name="y_sb", bufs=3) as ysb_pool:

                # HAM warmup: dummy matmuls during the input-load window so
                # the PE clock-gate is at 8/8 when real work arrives; also
                # pre-trigger the exp ACT table load off the critical path.
                warm = ppool.tile([128, 128], f16, tag="warm", bufs=1)
                warm2 = ppool.tile([128, 128], f16, tag="warm2", bufs=1)
                nc.vector.memset(warm[:], 1.0)
                nc.scalar.activation(warm2[:, 0:1], warm[:, 0:1], Exp)
                wps = ybcp.tile([128, 512], f32, tag="ybc", name="warm_ps")
                for _ in range(14):
                    nc.tensor.matmul(
                        wps[:], warm[:],
                        warm[:, 0:1].to_broadcast((128, 512)),
                        start=True, stop=True)

                # quarter-size projection chunks keep the filler
                # granularity near ~0.4us so the per-step drain slots pack
                # the PE slack left by the exp-paced attention pipeline
                NCH = 4

                def proj_qk_chunk(w_sb, dstT, fc, t5, ch, state):
                    if ch == 0:
                        state[fc] = ybcp.tile([128, 512], f32, tag="ybc",
                                              name=f"ps_{t5}_{fc}")
                    ps = state[fc]
                    per = DC // NCH
                    for dc in range(per * ch, per * ch + per):
                        nc.tensor.matmul(
                            ps[:],
                            w_sb[:, dc, fc * 128:(fc + 1) * 128],
                            xT[:, t5, dc, :],
                            start=(dc == 0), stop=(dc == DC - 1))
                    if ch == NCH - 1:
                        nc.vector.tensor_copy(
                            dstT[:, fc, t5 * 512:(t5 + 1) * 512], ps[:])

                def proj_qk(t5):
                    st = {}
                    for w_sb, dstT in ((wq, qT), (wk, kT)):
                        for fc in range(FC):
                            for ch in range(NCH):
                                proj_qk_chunk(w_sb, dstT, fc, t5, ch, st)

                def proj_v_chunk(tt, ch, state):
                    t5, r = divmod(tt, 4)
                    if ch == 0:
                        state[tt] = ybcp.tile([128, F], f32, tag="ybc",
                                              name=f"psv_{tt}")
                    psv = state[tt]
                    for dc in range(4 * ch, 4 * ch + 4):
                        nc.tensor.matmul(
                            psv[:],
                            xT[:, t5, dc, r * 128:(r + 1) * 128],
                            wv[:, dc, :],
                            start=(dc == 0), stop=(dc == DC - 1))
                    if ch == 1:
                        nc.vector.tensor_copy(
                            v[:, tt, :, 0:DH],
                            psv.rearrange("p (h d) -> p h d", h=HL))

                def proj_v_group(tt):
                    st = {}
                    proj_v_chunk(tt, 0, st)
                    proj_v_chunk(tt, 1, st)

                # two priority classes: "must" fillers gate the next
                # q-chunk's scores (projections); "soft" are elastic
                must = collections.deque()
                soft = collections.deque()

                def run_filler(n):
                    for _ in range(n):
                        if must:
                            must.popleft()()
                        elif soft:
                            soft.popleft()()

                def drain_must():
                    while must:
                        must.popleft()()

                def norm_pair(qc, hc):
                    # 1/l on the [q-partition] transposed copy (128 DVE
                    # lanes), broadcast over the dh rows with K=128 ident
                    # matmuls, two heads packed onto PE column halves,
                    # then one tensor_mul normalizes the [128,512] chunk
                    sl = slice(qc * 512, (qc + 1) * 512)
                    if qc == NQ - 1:
                        # on-chip l transpose: [1,128] -> [128,1] PE
                        # transposes into PSUM; skips the DRAM roundtrip
                        # latency that would sit on the critical tail
                        ltp = ybcp.tile([128, 8], f32, tag="ybc",
                                        name=f"ltp_{hc}")
                        for hp2 in range(2):
                            h = hc * 2 + hp2
                            for t4 in range(4):
                                seg = slice(h * S + qc * 512 + t4 * 128,
                                            h * S + qc * 512 + (t4 + 1) * 128)
                                nc.tensor.transpose(
                                    ltp[:, 4 * hp2 + t4:4 * hp2 + t4 + 1],
                                    l_row[0:1, seg], ones1[:])
                        with nc.allow_low_precision(
                                reason="fp16 1/l; l>=1 so ~5e-4 relative"):
                            for hp2 in range(2):
                                h = hc * 2 + hp2
                                cols = slice(h * NT + 4 * qc,
                                             h * NT + 4 * qc + 4)
                                nc.vector.reciprocal(
                                    recipT16[:, cols],
                                    ltp[:, 4 * hp2:4 * hp2 + 4])
                    else:
                        with nc.allow_low_precision(
                                reason="fp16 1/l; l>=1 so ~5e-4 relative"):
                            for hp2 in range(2):
                                h = hc * 2 + hp2
                                cols = slice(h * NT + 4 * qc,
                                             h * NT + 4 * qc + 4)
                                nc.vector.reciprocal(recipT16[:, cols],
                                                     lT[:, cols])
                    bc = ybcp.tile([128, 512], f32, tag="ybc",
                                   name=f"bc_{hc}_{qc}")
                    for hp2 in range(2):
                        for t4 in range(4):
                            col = (hc * 2 + hp2) * NT + 4 * qc + t4
                            nc.tensor.matmul(
                                bc[hp2 * 64:(hp2 + 1) * 64,
                                   t4 * 128:(t4 + 1) * 128],
                                recipT16[:, col:col + 1]
                                .to_broadcast((128, DH)),
                                ident[:],
                                start=True, stop=True,
                                tile_position=(0, hp2 * 64))
                    nc.vector.tensor_mul(
                        outT[:, hc, sl], outT[:, hc, sl], bc[:])

                def wo_qt(qt, tail=False):
                    ysb = ysb_pool.tile([128, 1024], f16, tag="ysb",
                                        name=f"ysb_{qt}")
                    for oc in range(2):
                        yps = ybcp.tile([128, 512], f32, tag="ybc",
                                        name=f"yps_{qt}_{oc}")
                        for fc in range(FC):
                            nc.tensor.matmul(
                                yps[:],
                                outT[:, fc, qt * 128:(qt + 1) * 128],
                                wo[:, fc, oc * 512:(oc + 1) * 512],
                                start=(fc == 0), stop=(fc == FC - 1))
                        dst = ysb[:, oc * 512:(oc + 1) * 512]
                        if tail and oc == 0:
                            # scalar is idle once the exps are done; split
                            # the tail casts so Vector isn't the pacer
                            nc.scalar.copy(dst, yps[:])
                        else:
                            nc.vector.tensor_copy(dst, yps[:])
                    nc.sync.dma_start(
                        y_d[qt * 128:(qt + 1) * 128, :], ysb[:])

                # ---- attention step pipeline ---------------------------
                # one step = one 128-key block for BOTH heads of the pair:
                # 2 score matmuls (PE row-halves) into disjoint column
                # halves of ONE [128,1024] PSUM tile, one exp ACT over a
                # [128,2,w] view, mask on diagonal, 2 AV matmuls.  Scores
                # are issued one step AHEAD of the exp/AV work, and the
                # 2-deep sc pool holds exactly the in-flight step + the
                # lookahead step, so the exp engine (the attention pacer)
                # never waits on the Tensor queue.
                steps = []
                pre_fill = {}     # step idx -> (must list, soft list)
                drain_before = set()

                def build_unit(qc, hc):
                    state = {}

                    def ensure_avs():
                        if 'avs' not in state:
                            state['avs'] = [
                                avp.tile([DH + 1, 512], f32, tag="av",
                                         name=f"av_{hc}_{qc}_{hp2}")
                                for hp2 in range(2)]
                        return state['avs']

                    def finalize_early():
                        # columns [0:384] are final once kb = n_kb-2 is
                        # accumulated (the last, r=3 block only writes
                        # [384:512]); copy them under the last AV
                        avs = state['avs']
                        for hp2 in range(2):
                            h = hc * 2 + hp2
                            nc.vector.tensor_copy(
                                outT[hp2 * 64:hp2 * 64 + 64, hc,
                                     qc * 512:qc * 512 + 384],
                                avs[hp2][0:DH, 0:384])
                            seg = slice(h * S + qc * 512,
                                        h * S + qc * 512 + 384)
                            nc.vector.tensor_copy(
                                l_row[0:1, seg], avs[hp2][DH:DH + 1, 0:384])

                    def finalize():
                        # one small staging read releases the avs PSUM pair
                        # (~0.26us after its last AV); the outT/l fan-out
                        # copies run off the release path
                        avs = state['avs']
                        for hp2 in range(2):
                            h = hc * 2 + hp2
                            stg = ppool.tile([DH + 1, 128], f32,
                                             tag="stg", bufs=2,
                                             name=f"stg_{hc}_{qc}_{hp2}")
                            nc.vector.tensor_copy(stg[:],
                                                  avs[hp2][:, 384:512])
                            nc.vector.tensor_copy(
                                outT[hp2 * 64:hp2 * 64 + 64, hc,
                                     qc * 512 + 384:(qc + 1) * 512],
                                stg[0:DH, :])
                            seg = slice(h * S + qc * 512 + 384,
                                        h * S + (qc + 1) * 512)
                            nc.vector.tensor_copy(
                                l_row[0:1, seg], stg[DH:DH + 1, :])
                            if qc < NQ - 1:
                                seg = slice(h * S + qc * 512,
                                            h * S + (qc + 1) * 512)
                                nc.sync.dma_start(l_dram[seg],
                                                  l_row[0:1, seg])
                                nc.sync.dma_start(
                                    lT[:, h * NT + 4 * qc:
                                       h * NT + 4 * qc + 4],
                                    l_dram[seg]
                                    .rearrange("(t p) -> p t", p=128))

                    n_kb = 4 * (qc + 1)
                    for kb in range(n_kb):
                        r = kb - 4 * qc
                        if r >= 0:
                            qo, w = 128 * r, 512 - 128 * r
                        else:
                            qo, w = 0, 512

                        def scores_fn(kb=kb, qo=qo, w=w):
                            sc = scp.tile([128, 1024], f32, tag="sc",
                                          name=f"sc_{hc}_{qc}_{kb}")
                            for hp2 in range(2):
                                hp = hp2 * 64
                                nc.tensor.matmul(
                                    sc[:, hp2 * 512 + qo:
                                       hp2 * 512 + qo + w],
                                    kT[hp:hp + 64, hc,
                                       kb * 128:(kb + 1) * 128],
                                    qT[hp:hp + 64, hc,
                                       qc * 512 + qo:(qc + 1) * 512],
                                    start=True, stop=True,
                                    tile_position=(hp, 0))
                            state[kb] = sc

                        def rest_fn(kb=kb, qo=qo, w=w, diag=(r >= 0),
                                    last=(kb == n_kb - 1)):
                            sc = state.pop(kb)
                            avs = ensure_avs()
                            p_sb = ppool.tile([128, 1024], f16, tag="p",
                                              name=f"p_{hc}_{qc}_{kb}")
                            sc2 = sc.rearrange("p (h w) -> p h w", h=2)
                            p2 = p_sb.rearrange("p (h w) -> p h w", h=2)
                            # the reference 1/sqrt(64) score scale
                            nc.scalar.activation(
                                p2[:, :, qo:qo + w],
                                sc2[:, :, qo:qo + w], Exp,
                                scale=0.125)
                            if diag:
                                # only the first 128 columns of a clipped
                                # block straddle the diagonal
                                for hp2 in range(2):
                                    o = hp2 * 512 + qo
                                    nc.vector.tensor_mul(
                                        p_sb[:, o:o + 128],
                                        p_sb[:, o:o + 128],
                                        mask[:, 384:512])
                            for hp2 in range(2):
                                h = hc * 2 + hp2
                                nc.tensor.matmul(
                                    avs[hp2][:, qo:512],
                                    v[:, kb, h, :],
                                    p_sb[:, hp2 * 512 + qo:
                                         hp2 * 512 + qo + w],
                                    start=(kb == 0),
                                    stop=(kb == n_kb - 1))
                            if kb == n_kb - 2:
                                finalize_early()
                            if last:
                                finalize()

                        steps.append((scores_fn, rest_fn))

                for qc in range(NQ):
                    idx = len(steps)
                    m, s = [], []
                    if qc + 1 < NQ:
                        st = {}
                        for w_sb, dstT in ((wq, qT), (wk, kT)):
                            for fc in range(FC):
                                for ch in range(NCH):
                                    m.append(
                                        lambda w=w_sb, d=dstT, f=fc,
                                        t=qc + 1, c=ch, stt=st:
                                        proj_qk_chunk(w, d, f, t, c, stt))
                        stv = {}
                        for tt in range(4 * (qc + 1), 4 * (qc + 2)):
                            for ch in range(2):
                                m.append(lambda t=tt, c=ch, stt=stv:
                                         proj_v_chunk(t, c, stt))
                    if qc >= 1:
                        for hcx in range(FC):
                            s.append(
                                lambda q=qc - 1, c=hcx: norm_pair(q, c))
                        for qt in range(4 * (qc - 1), 4 * qc):
                            s.append(lambda a=qt: wo_qt(a))
                    pre_fill[idx] = (m, s)
                    if qc > 0:
                        drain_before.add(idx)
                    build_unit(qc, 0)
                    # norm for (qc,0) of the last chunk runs as a filler
                    # inside (qc,1) so only (qc,1)'s norm sits on the tail
                    if qc == NQ - 1:
                        pre_fill[len(steps)] = (
                            [], [lambda: norm_pair(NQ - 1, 0)])
                    build_unit(qc, 1)

                proj_qk(0)
                proj_v_group(0)
                stv0 = {}
                for tt in (1, 2, 3):
                    for ch in range(2):
                        must.append(lambda t=tt, c=ch, stt=stv0:
                                    proj_v_chunk(t, c, stt))

                def issue_scores(k):
                    if k in pre_fill:
                        m, s = pre_fill[k]
                        must.extend(m)
                        soft.extend(s)
                    if k in drain_before:
                        drain_must()
                    steps[k][0]()

                issue_scores(0)
                for k in range(len(steps)):
                    if k + 1 < len(steps):
                        issue_scores(k + 1)
                    steps[k][1]()
                    if k == 12:
                        # stage the straggler loads behind real deps: a
                        # 1-element copy from qT stripe 1 (emitted by the
                        # qc=1 boundary drain, before this point) into each
                        # DMA's dst tile makes the scheduler hold the
                        # transfer until the early loads have drained the
                        # rings (they round-robin bandwidth otherwise)
                        for dst in (xT[0:1, 2, 0, 0:1],
                                    xT[0:1, 3, 0, 0:1],
                                    wo[0:1, 0, 0:1]):
                            nc.vector.tensor_copy(dst,
                                                  qT[0:1, 0, 512:513])
                        nc.sync.dma_start(xT[:, 2], xT_d[2])
                        nc.sync.dma_start(xT[:, 3], xT_d[3])
                        nc.scalar.dma_start(wo[:], wo_d[:])
                    run_filler(3 if k < 8 else 2)
                while must or soft:
                    run_filler(1)

                norm_pair(NQ - 1, 1)
                for qt in range(4 * (NQ - 1), 4 * NQ):
                    wo_qt(qt, tail=True)

    nc.compile()

    from concourse.bass_interp import get_hw_module
    nc.m = get_hw_module(nc.m)

    _CACHE[key] = nc
    return nc


def _make_mask():
    # mask[p, j] = 1 where (j - p) >= 384; slices of width 512 at offset
    # 384-128*r give the causal mask for a diagonal block at relative
    # position r (k block kb = 4*qc + r vs the 512-wide q chunk qc)
    j = np.arange(896)[None, :]
    p = np.arange(128)[:, None]
    return ((j - p) >= 384).astype(np.float16)


def kernel(x, wq, wk, wv, wo):
    x = np.asarray(x, dtype=np.float32)
    wq = np.asarray(wq, dtype=np.float32)
    wk = np.asarray(wk, dtype=np.float32)
    wv = np.asarray(wv, dtype=np.float32)
    wo = np.asarray(wo, dtype=np.float32)

    from concourse import bass_utils

    nc = _build_program()
    mask = _make_mask()

    def sbuf_w(w):
        # [out=256, in=1024] -> [128, DC, 256] SBUF layout, contiguous DMA
        return np.ascontiguousarray(
            w.T.reshape(DC, 128, F).transpose(1, 0, 2)).astype(np.float16)

    in_maps = []
    for c in range(8):
        b = c // 4
        hg = c % 4
        fs = slice(hg * F, (hg + 1) * F)
        # [NQ, 128, DC, 512]: stripe-major for early projection start
        xT = np.ascontiguousarray(
            x[b].T.reshape(DC, 128, NQ, 512).transpose(2, 1, 0, 3)
        ).astype(np.float16)
        woT = np.ascontiguousarray(
            wo[:, fs].T.reshape(FC, 128, D).transpose(1, 0, 2)
        ).astype(np.float16)
        in_maps.append({
            "xT": xT,
            "wq": sbuf_w(wq[fs, :]),
            "wk": sbuf_w(wk[fs, :]),
            "wv": sbuf_w(wv[fs, :]),
            "wo": woT,
            "mask": mask,
            "ident": np.eye(128, dtype=np.float16),
        })

    res = bass_utils.run_bass_kernel_spmd(nc, in_maps, core_ids=list(range(8)))
    ys = [res.results[c]["y"].astype(np.float32) for c in range(8)]
    out = np.stack([ys[0] + ys[1] + ys[2] + ys[3],
                    ys[4] + ys[5] + ys[6] + ys[7]])
    return out


# revision 28
# speedup vs baseline: 1.1800x; 1.0133x over previous
"""Trainium2 Bass kernel for a 16-head causal MHA layer.

Problem: x:[2,2048,1024] f32, wq/wk/wv/wo:[1024,1024] f32 (Linear-style
[out,in] weights), causal softmax attention with 16 heads of dim 64.

Sharding across the 8 NeuronCores: 2-way data parallel over batch x
4-way tensor parallel over heads.  Core c handles batch c//4 and the 4
heads 4*(c%4) .. 4*(c%4)+3 (feature slice of 256 rows of wq/wk/wv and
256 columns of wo).  Each core produces a partial [2048,1024] output
(its 4 heads' contribution, already projected through its wo slice);
the host sums the 4 partials per batch.

Device dataflow (all matmul inputs fp16, fp32 PSUM accumulation; fp8
was tried and measured 2.1% l2 error -- the softmax does not attenuate
relative error since the attention output shrinks by the same sqrt(eff_k)
factor as the injected score noise -- so everything stays fp16):
  - x arrives fp16, striped by 512-token chunks so projections start as
    soon as the first stripe lands
  - qT/kT = W @ xT in [feat, token] layout; the reference 1/sqrt(64)
    score scale is folded into the exp activation's free scale slot
  - scoresT[k,q] = kT_h.T-block @ qT_h (64-dim contraction, two heads
    packed onto PE row-halves via tile_position), exp on ACT straight
    out of PSUM, causal mask applied only on diagonal blocks via a
    precomputed 0/1 mask multiply
  - out_unnorm.T | l = (v|1).T-block @ expT accumulated over k blocks
    (the appended ones-column yields the softmax denominator l for free)
  - l stays on-chip in a [2, hc, S] row tile; DVE reciprocal, then a
    K=2 selector matmul broadcasts 1/l across the 128 outT partitions
    (row-half per head), one tensor_mul normalizes both heads at once
  - y = outT.T @ woT accumulated over the 256-dim feature slice; the
    two 512-wide output halves share one [128,1024] staging tile and a
    single DMA per 128-token row block
"""

import numpy as np

S = 2048          # sequence length (one batch per core)
D = 1024          # model dim
HL = 4            # heads handled per core
DH = 64           # head dim
F = HL * DH       # 256 local features
DC = D // 128     # 8 d_model chunks of 128
FC = F // 128     # 2 feature chunks of 128
NT = S // 128     # 16 token tiles
NQ = S // 512     # 4 query chunks of 512

_CACHE = {}


def _build_program(dbg=False):
    key = ("nc", dbg)
    if key in _CACHE:
        return _CACHE[key]

    import concourse.bacc as bacc
    import concourse.bass as bass
    import concourse.mybir as mybir
    import concourse.tile as tile

    f16 = mybir.dt.float16
    f32 = mybir.dt.float32
    Exp = mybir.ActivationFunctionType.Exp

    nc = bacc.Bacc("TRN2", target_bir_lowering=False, debug=False)

    # x striped by 512-token chunks: xT_d[t5][p, dc, j] = x[t5*512+j, dc*128+p]
    xT_d = nc.dram_tensor("xT", [NQ, 128, DC, 512], f16, kind="ExternalInput")
    wq_d = nc.dram_tensor("wq", [128, DC, F], f16, kind="ExternalInput")
    wk_d = nc.dram_tensor("wk", [128, DC, F], f16, kind="ExternalInput")
    wv_d = nc.dram_tensor("wv", [128, DC, F], f16, kind="ExternalInput")
    wo_d = nc.dram_tensor("wo", [128, FC, D], f16, kind="ExternalInput")
    mask_d = nc.dram_tensor("mask", [128, 896], f16, kind="ExternalInput")
    ident_d = nc.dram_tensor("ident", [128, 128], f16, kind="ExternalInput")
    y_d = nc.dram_tensor("y", [S, D], f16, kind="ExternalOutput")

    with tile.TileContext(nc) as tc:
        with tc.tile_pool(name="const", bufs=1) as cpool, \
             tc.tile_pool(name="dscr", bufs=1,
                          space=bass.MemorySpace.DRAM) as dpool:
            l_dram = dpool.tile([HL * S], f32)
            xT = cpool.tile([128, NQ, DC, 512], f16)
            wq = cpool.tile([128, DC, F], f16)
            wk = cpool.tile([128, DC, F], f16)
            wv = cpool.tile([128, DC, F], f16)
            wo = cpool.tile([128, FC, D], f16)
            mask = cpool.tile([128, 896], f16)
            ident = cpool.tile([128, 128], f16)
            qT = cpool.tile([128, FC, S], f16)
            kT = cpool.tile([128, FC, S], f16)
            v = cpool.tile([128, NT, HL, DH + 1], f16)
            outT = cpool.tile([128, FC, S], f16)
            l_row = cpool.tile([1, HL * S], f32)
            lT = cpool.tile([128, HL * NT], f32)
            recipT16 = cpool.tile([128, HL * NT], f16)

            # loads: sync + scalar are the only hardware-DGE queues.
            # scalar gets only the early small loads (it becomes exp-bound);
            # everything else streams on sync in first-needed order.
            nc.sync.dma_start(xT[:, 0], xT_d[0])
            nc.scalar.dma_start(wq[:], wq_d[:])
            nc.scalar.dma_start(wk[:], wk_d[:])
            nc.scalar.dma_start(wv[:], wv_d[:])
            nc.scalar.dma_start(mask[:], mask_d[:])
            nc.scalar.dma_start(ident[:], ident_d[:])
            nc.sync.dma_start(xT[:, 1], xT_d[1])
            nc.sync.dma_start(xT[:, 2], xT_d[2])
            nc.sync.dma_start(xT[:, 3], xT_d[3])
            nc.scalar.dma_start(wo[:], wo_d[:])

            # constants / ones columns for the softmax-denominator trick
            nc.gpsimd.memset(v[:], 1.0)

            with tc.tile_pool(name="sc_ps", bufs=2,
                              space=bass.MemorySpace.PSUM) as scp, \
                 tc.tile_pool(name="av_ps", bufs=2,
                              space=bass.MemorySpace.PSUM) as avp, \
                 tc.tile_pool(name="ybc_ps", bufs=2,
                              space=bass.MemorySpace.PSUM) as ybcp, \
                 tc.tile_pool(name="p_sb", bufs=6) as ppool, \
                 tc.tile_pool(name="y_sb", bufs=3) as ysb_pool:

                # HAM warmup: dummy matmuls during the input-load window so
                # the PE clock-gate is at 8/8 when real work arrives; also
                # pre-trigger the exp ACT table load off the critical path.
                warm = ppool.tile([128, 128], f16, tag="warm", bufs=1)
                warm2 = ppool.tile([128, 128], f16, tag="warm2", bufs=1)
                nc.vector.memset(warm[:], 1.0)
                nc.scalar.activation(warm2[:, 0:1], warm[:, 0:1], Exp)
                wps = ybcp.tile([128, 512], f32, tag="ybc", name="warm_ps")
                for _ in range(24):
                    nc.tensor.matmul(
                        wps[:], warm[:],
                        warm[:, 0:1].to_broadcast((128, 512)),
                        start=True, stop=True)

                def proj_qk_group(w_sb, dstT, fc, t5):
                    ps = ybcp.tile([128, 512], f32, tag="ybc",
                                   name=f"ps_{t5}_{fc}")
                    for dc in range(DC):
                        nc.tensor.matmul(
                            ps[:],
                            w_sb[:, dc, fc * 128:(fc + 1) * 128],
                            xT[:, t5, dc, :],
                            start=(dc == 0), stop=(dc == DC - 1))
                    nc.vector.tensor_copy(
                        dstT[:, fc, t5 * 512:(t5 + 1) * 512], ps[:])

                def proj_qk(t5):
                    for w_sb, dstT in ((wq, qT), (wk, kT)):
                        for fc in range(FC):
                            proj_qk_group(w_sb, dstT, fc, t5)

                def proj_v_group(tt):
                    t5, r = divmod(tt, 4)
                    psv = ybcp.tile([128, F], f32, tag="ybc",
                                    name=f"psv_{tt}")
                    for dc in range(DC):
                        nc.tensor.matmul(
                            psv[:],
                            xT[:, t5, dc, r * 128:(r + 1) * 128],
                            wv[:, dc, :],
                            start=(dc == 0), stop=(dc == DC - 1))
                    nc.vector.tensor_copy(
                        v[:, tt, :, 0:DH],
                        psv.rearrange("p (h d) -> p h d", h=HL))

                import collections
                fillers = collections.deque()

                def run_filler(n):
                    for _ in range(n):
                        if fillers:
                            fillers.popleft()()

                def att_hc(qc, hc):
                    avs = []
                    for hp2 in range(2):
                        av = avp.tile([DH + 1, 512], f32, tag="av",
                                      name=f"av_{hc}_{qc}_{hp2}")
                        avs.append(av)
                    for g in range(qc + 1):
                        diag = (g == qc)
                        for half in range(2):
                            # (offset, width) of each k-block's valid
                            # q-span inside the p tile; diagonal blocks
                            # are clipped to q >= k_block_start
                            if diag:
                                rs = [2 * half, 2 * half + 1]
                                spans = [(128 * r, 512 - 128 * r)
                                         for r in rs]
                            else:
                                spans = [(0, 512), (0, 512)]
                            offs = [0, spans[0][1]]
                            scs = []
                            for hp2 in range(2):
                                sc = scp.tile([128, 1024], f32, tag="sc",
                                              name=f"sc_{hc}_{qc}_{g}_{half}_{hp2}")
                                scs.append(sc)
                            for r2 in range(2):
                                kb = 4 * g + 2 * half + r2
                                qo, w = spans[r2]
                                for hp2 in range(2):
                                    hp = hp2 * 64
                                    nc.tensor.matmul(
                                        scs[hp2][:, offs[r2]:offs[r2] + w],
                                        kT[hp:hp + 64, hc,
                                           kb * 128:(kb + 1) * 128],
                                        qT[hp:hp + 64, hc,
                                           qc * 512 + qo:(qc + 1) * 512],
                                        start=True, stop=True,
                                        tile_position=(hp, 0))
                            width = offs[1] + spans[1][1]
                            for hp2 in range(2):
                                h = hc * 2 + hp2
                                p_sb = ppool.tile([128, 1024], f16,
                                                  tag=f"p{hp2}",
                                                  name=f"p_{hc}_{qc}_{g}_{half}_{hp2}")
                                # the reference 1/sqrt(64) score scale
                                nc.scalar.activation(
                                    p_sb[:, 0:width],
                                    scs[hp2][:, 0:width], Exp,
                                    scale=0.125)
                                if diag:
                                    # only the first 128 columns of a
                                    # clipped block straddle the diagonal
                                    for r2 in range(2):
                                        nc.vector.tensor_mul(
                                            p_sb[:, offs[r2]:offs[r2] + 128],
                                            p_sb[:, offs[r2]:offs[r2] + 128],
                                            mask[:, 384:512])
                                for r2 in range(2):
                                    kb = 4 * g + 2 * half + r2
                                    qo, w = spans[r2]
                                    nc.tensor.matmul(
                                        avs[hp2][:, qo:512],
                                        v[:, kb, h, :],
                                        p_sb[:, offs[r2]:offs[r2] + w],
                                        start=(kb == 0),
                                        stop=(kb == 4 * qc + 3))
                            run_filler(2)
                    for hp2 in range(2):
                        h = hc * 2 + hp2
                        nc.vector.tensor_copy(
                            outT[hp2 * 64:hp2 * 64 + 64, hc,
                                 qc * 512:(qc + 1) * 512],
                            avs[hp2][0:DH, :])
                        # denominators: roundtrip through DRAM to land the
                        # 512 l values across 128 partitions (a [1,512]
                        # single-lane DVE reciprocal measures 3.3us; the
                        # [128,4] one is ~0.15us)
                        seg = slice(h * S + qc * 512,
                                    h * S + (qc + 1) * 512)
                        nc.vector.tensor_copy(
                            l_row[0:1, seg], avs[hp2][DH:DH + 1, :])
                        nc.sync.dma_start(l_dram[seg], l_row[0:1, seg])
                        nc.sync.dma_start(
                            lT[:, h * NT + 4 * qc:h * NT + 4 * qc + 4],
                            l_dram[seg].rearrange("(t p) -> p t", p=128))

                def norm_pair(qc, hc):
                    # 1/l on the [q-partition] transposed copy (128 DVE
                    # lanes), broadcast over the dh rows with K=128 ident
                    # matmuls -- the two heads packed onto PE column halves
                    # via tile_position -- then one tensor_mul normalizes
                    # the whole [128,512] chunk
                    sl = slice(qc * 512, (qc + 1) * 512)
                    with nc.allow_low_precision(
                            reason="fp16 1/l; l>=1 so ~5e-4 relative"):
                        for hp2 in range(2):
                            h = hc * 2 + hp2
                            cols = slice(h * NT + 4 * qc,
                                         h * NT + 4 * qc + 4)
                            nc.vector.reciprocal(recipT16[:, cols],
                                                 lT[:, cols])
                    bc = ybcp.tile([128, 512], f32, tag="ybc",
                                   name=f"bc_{hc}_{qc}")
                    for hp2 in range(2):
                        for t4 in range(4):
                            col = (hc * 2 + hp2) * NT + 4 * qc + t4
                            nc.tensor.matmul(
                                bc[hp2 * 64:(hp2 + 1) * 64,
                                   t4 * 128:(t4 + 1) * 128],
                                recipT16[:, col:col + 1]
                                .to_broadcast((128, DH)),
                                ident[:],
                                start=True, stop=True,
                                tile_position=(0, hp2 * 64))
                    nc.vector.tensor_mul(
                        outT[:, hc, sl], outT[:, hc, sl], bc[:])

                def wo_qt(qt):
                    ysb = ysb_pool.tile([128, 1024], f16, tag="ysb",
                                        name=f"ysb_{qt}")
                    for oc in range(2):
                        yps = ybcp.tile([128, 512], f32, tag="ybc",
                                        name=f"yps_{qt}_{oc}")
                        for fc in range(FC):
                            nc.tensor.matmul(
                                yps[:],
                                outT[:, fc, qt * 128:(qt + 1) * 128],
                                wo[:, fc, oc * 512:(oc + 1) * 512],
                                start=(fc == 0), stop=(fc == FC - 1))
                        nc.vector.tensor_copy(
                            ysb[:, oc * 512:(oc + 1) * 512], yps[:])
                    nc.sync.dma_start(
                        y_d[qt * 128:(qt + 1) * 128, :], ysb[:])

                proj_qk(0)
                for tt in range(4):
                    proj_v_group(tt)
                for qc in range(NQ):
                    if qc + 1 < NQ:
                        for w_sb, dstT in ((wq, qT), (wk, kT)):
                            for fc in range(FC):
                                fillers.append(
                                    lambda w=w_sb, d=dstT, f=fc, t=qc + 1:
                                    proj_qk_group(w, d, f, t))
                        for tt in range(4 * (qc + 1), 4 * (qc + 2)):
                            fillers.append(lambda t=tt: proj_v_group(t))
                    if qc >= 1:
                        for hcx in range(FC):
                            fillers.append(
                                lambda q=qc - 1, c=hcx: norm_pair(q, c))
                        for qt in range(4 * (qc - 1), 4 * qc):
                            fillers.append(lambda a=qt: wo_qt(a))
                    att_hc(qc, 0)
                    att_hc(qc, 1)
                    run_filler(len(fillers))
                norm_pair(NQ - 1, 0)
                norm_pair(NQ - 1, 1)
                for qt in range(4 * (NQ - 1), 4 * NQ):
                    wo_qt(qt)

    nc.compile()

    from concourse.bass_interp import get_hw_module
    nc.m = get_hw_module(nc.m)

    _CACHE[key] = nc
    return nc


def _make_mask():
    # mask[p, j] = 1 where (j - p) >= 384; slices of width 512 at offset
    # 384-128*r give the causal mask for a diagonal block at relative
    # position r (k block kb = 4*qc + r vs the 512-wide q chunk qc)
    j = np.arange(896)[None, :]
    p = np.arange(128)[:, None]
    return ((j - p) >= 384).astype(np.float16)


def kernel(x, wq, wk, wv, wo):
    x = np.asarray(x, dtype=np.float32)
    wq = np.asarray(wq, dtype=np.float32)
    wk = np.asarray(wk, dtype=np.float32)
    wv = np.asarray(wv, dtype=np.float32)
    wo = np.asarray(wo, dtype=np.float32)

    from concourse import bass_utils

    nc = _build_program()
    mask = _make_mask()

    def sbuf_w(w):
        # [out=256, in=1024] -> [128, DC, 256] SBUF layout, contiguous DMA
        return np.ascontiguousarray(
            w.T.reshape(DC, 128, F).transpose(1, 0, 2)).astype(np.float16)

    in_maps = []
    for c in range(8):
        b = c // 4
        hg = c % 4
        fs = slice(hg * F, (hg + 1) * F)
        # [NQ, 128, DC, 512]: stripe-major for early projection start
        xT = np.ascontiguousarray(
            x[b].T.reshape(DC, 128, NQ, 512).transpose(2, 1, 0, 3)
        ).astype(np.float16)
        woT = np.ascontiguousarray(
            wo[:, fs].T.reshape(FC, 128, D).transpose(1, 0, 2)
        ).astype(np.float16)
        in_maps.append({
            "xT": xT,
            "wq": sbuf_w(wq[fs, :]),
            "wk": sbuf_w(wk[fs, :]),
            "wv": sbuf_w(wv[fs, :]),
            "wo": woT,
            "mask": mask,
            "ident": np.eye(128, dtype=np.float16),
        })

    res = bass_utils.run_bass_kernel_spmd(nc, in_maps, core_ids=list(range(8)))
    ys = [res.results[c]["y"].astype(np.float32) for c in range(8)]
    out = np.stack([ys[0] + ys[1] + ys[2] + ys[3],
                    ys[4] + ys[5] + ys[6] + ys[7]])
    return out
